# revision 10
# baseline (speedup 1.0000x reference)
"""MicroVoxelSpatialEncoder Trainium2 kernel.

Strategy (8 NeuronCores, no collectives):
  - Host (numpy, integer/index work only): voxel hashing, per-batch sort of
    points by voxel id, segment/selection matrices, neighbor gather indices,
    validity masks.
  - Device phase A (replicated on every core): build a per-voxel table of
    pre-projected K/V rows (fp16, [K|V] = 512B per voxel) in DRAM.
    Segment means are computed with PE selection matmuls (1/cnt folded into
    the selection matrix), the intra-voxel MLP + K/V projections are PE
    matmuls, positional-encoding MLP included.
  - Device phase B (sharded: core c owns flat points [c*1024,(c+1)*1024)):
    dma_gather 27 neighbor rows per point, masked per-point multi-head
    attention, output projection, residual + layernorm.
"""

import os
import sys

import numpy as np

for _p in ("/opt/trn_rl_repo", "/root/.axon_site/_ro/trn_rl_repo"):
    if _p not in sys.path and os.path.isdir(_p):
        sys.path.append(_p)

import concourse.bacc as bacc
import concourse.bass as bass
import concourse.tile as tile
from concourse import bass_utils, mybir
from concourse.masks import make_identity

F32 = mybir.dt.float32
F32R = mybir.dt.float32r
F16 = mybir.dt.float16
I16 = mybir.dt.int16

GRID = (64, 64, 100)
V_GRID = GRID[0] * GRID[1] * GRID[2]
MINS = np.array([0.0, 0.0, 0.0], np.float32)
MAXS = np.array([256.0, 256.0, 1.0], np.float32)
H = 4
LN_EPS = 1e-5

P = 128
B, N, DIN, D = 2, 4096, 32, 128
DH = D // H
K27 = 27
NCORES = 8
PTS_CORE = (B * N) // NCORES      # 1024
CHB = 4                            # phase-B chunks per core
PTS_CH = PTS_CORE // CHB           # 256
G2 = PTS_CH // P                   # 2
KG = K27 * G2                      # 54
NI = KG * P                        # 6912 gather rows per chunk
IDXW = NI // 16                    # 432

_cache = {}


# ----------------------------------------------------------------- host prep
def _voxelize(coords):
    """Exactly mirror the reference's fp32 voxel hashing."""
    c = coords.astype(np.float32)
    norm = np.clip((c - MINS) / (MAXS - MINS), np.float32(0.0), np.float32(1.0))
    g = np.array(GRID, np.int32)
    vidx = (norm * (g - 1).astype(np.float32)).astype(np.int32)
    vid = (vidx[..., 0] * g[1] + vidx[..., 1]) * g[2] + vidx[..., 2]
    return vidx, vid


def _host_prep(features, coords):
    vidx, vid = _voxelize(coords)

    # --- per-batch sort by voxel id; chunk so no voxel-run spans a chunk ---
    slot_maps = []      # per batch: dict vid -> slot
    chunks = []         # list of (batch, [(sorted_point_indices, r)] per chunk)
    for b in range(B):
        order = np.argsort(vid[b], kind="stable")
        svid = vid[b][order]
        uniq, starts, cnts = np.unique(svid, return_index=True, return_counts=True)
        if cnts.max() > P:
            raise NotImplementedError("voxel with more than 128 points")
        runs = list(zip(uniq.tolist(), starts.tolist(), cnts.tolist()))
        cur = []  # list of runs in current chunk
        cur_n = 0
        bchunks = []
        for u, s, c in runs:
            if cur_n + c > P:
                bchunks.append(cur)
                cur, cur_n = [], 0
            cur.append((u, s, c))
            cur_n += c
        if cur:
            bchunks.append(cur)
        chunks.append((b, bchunks, order))
        slot_maps.append({})

    ca_list = [len(chunks[b][1]) for b in range(B)]
    CA = sum(ca_list)
    CA = -(-CA // 4) * 4  # pad total chunk count to a multiple of 4
    # distribute padding chunks (all-dummy) at the end
    R_TAB = CA * P + 16   # table rows: slots + dummy row at CA*P (rest pad)
    DUMMY = CA * P

    feats_flat = features.reshape(B * N, DIN).astype(np.float32)

    XS = np.zeros((P, CA, DIN + 1), np.float32)
    S = np.zeros((P, CA * P), np.float16)
    c_global = 0
    for b, bchunks, order in chunks:
        for runs in bchunks:
            p0 = 0
            for j, (u, s, c) in enumerate(runs):
                idxs = order[s : s + c]  # original point indices in batch b
                r = np.float16(1.0 / np.float32(c))
                for pi in idxs:
                    XS[p0, c_global, :DIN] = feats_flat[b * N + pi]
                    XS[p0, c_global, DIN] = 1.0
                    S[p0, c_global * P + j] = r
                    p0 += 1
                slot_maps[b][u] = c_global * P + j
            c_global += 1
    XS = XS.reshape(P, CA * (DIN + 1))

    # --- neighbor indices / masks per core ---
    rr = np.arange(-1, 2)
    offs = np.stack(np.meshgrid(rr, rr, rr, indexing="ij"), -1).reshape(-1, 3)
    g = np.array(GRID, np.int32)

    vidx_flat = vidx.reshape(B * N, 3)
    nidx = vidx_flat[:, None, :] + offs[None, :, :]          # (BN, 27, 3)
    inb = np.all((nidx >= 0) & (nidx < g), axis=-1)          # (BN, 27)
    ncl = np.clip(nidx, 0, g - 1)
    nvid = (ncl[..., 0] * g[1] + ncl[..., 1]) * g[2] + ncl[..., 2]

    j_all = np.full((B * N, K27), DUMMY, np.int64)
    valid = np.zeros((B * N, K27), bool)
    for b in range(B):
        sm = slot_maps[b]
        base = b * N
        for i in range(N):
            row = nvid[base + i]
            ib = inb[base + i]
            for k in range(K27):
                if ib[k]:
                    s = sm.get(int(row[k]))
                    if s is not None:
                        j_all[base + i, k] = s
                        valid[base + i, k] = True
    has = valid.any(-1)

    per_core = []
    for core in range(NCORES):
        sl = slice(core * PTS_CORE, (core + 1) * PTS_CORE)
        jc = j_all[sl].reshape(CHB, G2, P, K27)
        # gather order i = (k*G2+g)*P + p  for each chunk
        idx = np.zeros((CHB, K27, G2, P), np.int64)
        for ch in range(CHB):
            for k in range(K27):
                for gg in range(G2):
                    idx[ch, k, gg] = jc[ch, gg, :, k]
        idx = idx.reshape(CHB, NI)
        blocks = []
        for ch in range(CHB):
            blk = np.zeros((16, IDXW), np.int16)
            lin = idx[ch]
            blk[np.arange(NI) % 16, np.arange(NI) // 16] = lin.astype(np.int16)
            blocks.append(np.tile(blk, (8, 1)))
        IDX = np.concatenate(blocks, axis=1)  # (128, CHB*IDXW)

        vc = valid[sl].reshape(CHB, G2, P, K27)
        MASK = np.zeros((P, CHB * KG * H), np.float16)
        for ch in range(CHB):
            for k in range(K27):
                for gg in range(G2):
                    for hh in range(H):
                        MASK[:, (ch * KG + k * G2 + gg) * H + hh] = vc[ch, gg, :, k]
        HAS = np.zeros((P, CHB * G2), np.float32)
        hc = has[sl].reshape(CHB * G2, P)
        for gg in range(CHB * G2):
            HAS[:, gg] = hc[gg]
        XO = np.zeros((DIN + 1, PTS_CORE), np.float32)
        XO[:DIN] = feats_flat[sl].T
        XO[DIN] = 1.0
        per_core.append({"IDX": IDX, "MASK": MASK, "HAS": HAS, "XO": XO})

    return XS, S, CA, R_TAB, per_core


# ------------------------------------------------------------- device program
def _build(CA, R_TAB):
    nc = bacc.Bacc(
        "TRN2",
        target_bir_lowering=False,
        debug=False,
        enable_asserts=False,
        num_devices=1,
    )
    CAP = CA * P
    dt_in = {
        "XS": ([P, CA * (DIN + 1)], F32),
        "S": ([P, CAP], F16),
        "XO": ([DIN + 1, PTS_CORE], F32),
        "IDX": ([P, CHB * IDXW], I16),
        "MASK": ([P, CHB * KG * H], F16),
        "HAS": ([P, CHB * G2], F32),
        "fW_aug": ([DIN + 1, D], F32),
        "fW_augT": ([D, DIN + 1], F32),
        "aW": ([D, D], F32),
        "ab_row": ([1, D], F32),
        "WqT": ([D, D], F32),
        "WkT": ([D, D], F32),
        "WvT": ([D, D], F32),
        "out_wT": ([D, D], F32),
        "bq_row": ([1, D], F32),
        "bk_col": ([D, 1], F32),
        "bv_col": ([D, 1], F32),
        "out_b_row": ([1, D], F32),
        "pW1": ([3, D // 2], F32),
        "pb1_col": ([D // 2, 1], F32),
        "pW2": ([D // 2, D], F32),
        "pb2_col": ([D, 1], F32),
        "offs_fT": ([3, K27], F32),
        "gamma_row": ([1, D], F32),
        "beta_row": ([1, D], F32),
        "eps_col": ([D, 1], F32),
        "onehot32": ([1, DIN + 1], F32),
        "ones1f": ([1, P], F32),
        "ones1h": ([1, P], F16),
    }
    dram = {k: nc.dram_tensor(k, shp, dt, kind="ExternalInput") for k, (shp, dt) in dt_in.items()}
    out_d = nc.dram_tensor("OUT", [P, CHB * G2, D], F32, kind="ExternalOutput")
    table = nc.dram_tensor("tableKV", [R_TAB, 2 * D], F16, kind="Internal")
    posdram = nc.dram_tensor("posdram", [K27, 2 * D], F16, kind="Internal")

    from contextlib import ExitStack

    with tile.TileContext(nc) as tc:
        with ExitStack() as ctx, nc.allow_low_precision(
            "fp16 phase-B math; tolerance-checked vs fp32 ref"
        ):
            _emit(ctx, tc, nc, dram, out_d, table, posdram, CA)
    nc.compile()
    return nc


def _emit(ctx, tc, nc, dram, out_d, table, posdram, CA):
    CAP = CA * P
    W512 = CAP // 512

    const = ctx.enter_context(tc.tile_pool(name="const", bufs=1))

    # ---------- load persistent inputs ----------
    t_in = {}
    for k in dram:
        if k in ("XS", "S"):
            continue
        shp = list(dram[k].shape)
        tl = const.tile(shp, dram[k].dtype, tag=f"in_{k}")
        nc.sync.dma_start(tl[:], dram[k].ap())
        t_in[k] = tl

    ident16 = const.tile([P, P], F16, tag="ident16")
    make_identity(nc, ident16[:])
    neg60k = const.tile([P, KG * H], F16, tag="neg60k")
    nc.vector.memset(neg60k[:], -60000.0)

    # fp16 casts of weights (persistent)
    WkT16 = const.tile([D, D], F16, tag="WkT16")
    nc.vector.tensor_copy(WkT16[:], t_in["WkT"][:])
    WvT16 = const.tile([D, D], F16, tag="WvT16")
    nc.vector.tensor_copy(WvT16[:], t_in["WvT"][:])
    owT16 = const.tile([D, D], F16, tag="owT16")
    nc.vector.tensor_copy(owT16[:], t_in["out_wT"][:])
    outb16 = const.tile([1, D], F16, tag="outb16")
    nc.vector.tensor_copy(outb16[:], t_in["out_b_row"][:])

    # persistent phase-A outputs used by phase B
    posrep = const.tile([P, K27, 2 * D], F16, tag="posrep")
    featR = const.tile([P, CHB * G2, D], F32, tag="featR")
    qR = const.tile([P, CHB * G2, D], F16, tag="qR")
    gammarep = const.tile([P, D], F32, tag="gammarep")
    betarep = const.tile([P, D], F32, tag="betarep")

    psSm = ctx.enter_context(tc.tile_pool(name="psSm", bufs=2, space="PSUM"))
    psKV = ctx.enter_context(tc.tile_pool(name="psKV", bufs=2, space="PSUM"))

    # ---------- phase A ----------
    with tc.tile_pool(name="pha", bufs=1) as pha, \
         tc.tile_pool(name="psA", bufs=2, space="PSUM") as psA, \
         tc.tile_pool(name="psW", bufs=2, space="PSUM") as psW, \
         tc.tile_pool(name="stage", bufs=2) as stage:
        XS32 = pha.tile([P, CA * (DIN + 1)], F32, tag="XS32")
        nc.sync.dma_start(XS32[:], dram["XS"].ap())
        S16 = pha.tile([P, CAP], F16, tag="S16")
        nc.sync.dma_start(S16[:], dram["S"].ap())
        XS16 = pha.tile([P, CA * (DIN + 1)], F16, tag="XS16")
        nc.vector.tensor_copy(XS16[:], XS32[:])

        # W2_aug = fW_aug @ aW (+ ab row); Wq2_aug = fW_aug @ Wq.T (+ bq row)
        W2s16 = pha.tile([DIN + 1, D], F16, tag="W2s16")
        Wq2s = pha.tile([DIN + 1, D], F32, tag="Wq2s")
        for dst, rhs_w, brow, f16out in ((W2s16, "aW", "ab_row", True), (Wq2s, "WqT", "bq_row", False)):
            ps = psA.tile([DIN + 1, D], F32, tag="psA")
            nc.tensor.matmul(ps[:], t_in["fW_augT"][:], t_in[rhs_w][:], start=True, stop=False)
            nc.tensor.matmul(ps[:], t_in["onehot32"][:], t_in[brow][:], start=False, stop=True)
            nc.vector.tensor_copy(dst[:], ps[:])

        # segment sums (means: 1/cnt is folded into S): gsumA[a, u]
        gsumA = pha.tile([DIN + 1, CAP], F16, tag="gsumA")
        for c in range(CA):
            ps = psA.tile([DIN + 1, P], F32, tag="psA")
            lhs = XS16[:, c * (DIN + 1) : (c + 1) * (DIN + 1)]
            nc.tensor.matmul(ps[:], lhs, S16[:, c * P : (c + 1) * P], start=True, stop=True)
            if c % 2 == 0:
                nc.vector.tensor_copy(gsumA[:, c * P : (c + 1) * P], ps[:])
            else:
                nc.scalar.copy(gsumA[:, c * P : (c + 1) * P], ps[:])

        # grid MLP: gridT16[do, u] = relu(W2_aug.T @ gsumA)
        gridT16 = pha.tile([D, CAP], F16, tag="gridT16")
        for w in range(W512):
            ps = psW.tile([D, 512], F32, tag="psW")
            nc.tensor.matmul(
                ps[:], W2s16[:], gsumA[:, w * 512 : (w + 1) * 512], start=True, stop=True
            )
            nc.scalar.activation(gridT16[:, w * 512 : (w + 1) * 512], ps[:], mybir.ActivationFunctionType.Relu)

        # K/V tables -> DRAM (fused rows [K|V], fp16), 4 chunks per DMA
        for c4 in range(CA // 4):
            st = stage.tile([P, 4, 2 * D], F16, tag="stage")
            for cc in range(4):
                c = c4 * 4 + cc
                ps = psKV.tile([P, 2 * D], F32, tag="psKV")
                lhs = gridT16[:, c * P : (c + 1) * P]
                nc.tensor.matmul(ps[:, 0:D], lhs, WkT16[:], start=True, stop=True)
                nc.tensor.matmul(ps[:, D : 2 * D], lhs, WvT16[:], start=True, stop=True)
                if cc % 2 == 0:
                    nc.vector.tensor_copy(st[:, cc, :], ps[:])
                else:
                    nc.scalar.copy(st[:, cc, :], ps[:])
            dst = table.ap()[c4 * 4 * P : (c4 * 4 + 4) * P, :]
            dst = dst.rearrange("(c p) e -> p c e", p=P, c=4)
            nc.sync.dma_start(dst, st[:])
        zrow = stage.tile([16, 2 * D], F16, tag="zrow")
        nc.vector.memset(zrow[:], 0.0)
        nc.sync.dma_start(table.ap()[CAP : CAP + 16, :], zrow[:])

        # positional encodings -> posK/posV rows (with k/v input biases folded in)
        ps = psSm.tile([D // 2, K27], F32, tag="psSm")
        nc.tensor.matmul(ps[:], t_in["pW1"][:], t_in["offs_fT"][:], start=True, stop=True)
        h1 = pha.tile([D // 2, K27], F32, tag="h1")
        nc.scalar.activation(h1[:], ps[:], mybir.ActivationFunctionType.Relu, bias=t_in["pb1_col"][:])
        ps = psSm.tile([D, K27], F32, tag="psSm")
        nc.tensor.matmul(ps[:], t_in["pW2"][:], h1[:], start=True, stop=True)
        posT = pha.tile([D, K27], F32, tag="posT")
        nc.scalar.activation(posT[:], ps[:], mybir.ActivationFunctionType.Identity, bias=t_in["pb2_col"][:])

        posKV16 = pha.tile([D, 2, K27], F16, tag="posKV16")
        for i, (wname, bcol) in enumerate((("WkT", "bk_col"), ("WvT", "bv_col"))):
            ps = psSm.tile([D, K27], F32, tag="psSm")
            nc.tensor.matmul(ps[:], t_in[wname][:], posT[:], start=True, stop=True)
            nc.scalar.activation(posKV16[:, i, :], ps[:], mybir.ActivationFunctionType.Identity, bias=t_in[bcol][:])

        posrows = pha.tile([K27, 2 * D], F16, tag="posrows")
        for i in range(2):
            ps = psSm.tile([K27, P], F16, tag="psSm")
            nc.tensor.matmul(ps[:], posKV16[:, i, :], ident16[:], is_transpose=True, start=True, stop=True)
            nc.vector.tensor_copy(posrows[:, i * D : (i + 1) * D], ps[:])
        nc.sync.dma_start(posdram.ap()[:, :], posrows[:])
        posflat = pha.tile([1, K27 * 2 * D], F16, tag="posflat")
        nc.sync.dma_start(posflat[:], posdram.ap().rearrange("a b -> (a b)").unsqueeze(0))
        for k in range(K27):
            ps = psKV.tile([P, 2 * D], F32, tag="psKV")
            nc.tensor.matmul(ps[:], t_in["ones1h"][:], posflat[:, k * 2 * D : (k + 1) * 2 * D], start=True, stop=True)
            if k % 2 == 0:
                nc.vector.tensor_copy(posrep[:, k, :], ps[:])
            else:
                nc.scalar.copy(posrep[:, k, :], ps[:])

        # feat / q rows for this core's points
        for gg in range(CHB * G2):
            xo = t_in["XO"][:, gg * P : (gg + 1) * P]
            ps = psSm.tile([P, D], F32, tag="psSm")
            nc.tensor.matmul(ps[:], xo, t_in["fW_aug"][:], start=True, stop=True)
            nc.vector.tensor_copy(featR[:, gg, :], ps[:])
            ps2 = psSm.tile([P, D], F32, tag="psSm")
            nc.tensor.matmul(ps2[:], xo, Wq2s[:], start=True, stop=True)
            nc.scalar.activation(
                qR[:, gg, :], ps2[:], mybir.ActivationFunctionType.Copy, bias=0.0, scale=float(1.0 / np.sqrt(DH))
            )

        for dst, row in ((gammarep, "gamma_row"), (betarep, "beta_row")):
            ps = psSm.tile([P, D], F32, tag="psSm")
            nc.tensor.matmul(ps[:], t_in["ones1f"][:], t_in[row][:], start=True, stop=True)
            nc.vector.tensor_copy(dst[:], ps[:])

    # ---------- phase B ----------
    gpool = ctx.enter_context(tc.tile_pool(name="gpool", bufs=2))
    bpool = ctx.enter_context(tc.tile_pool(name="bpool", bufs=2))
    spool = ctx.enter_context(tc.tile_pool(name="spool", bufs=2))

    for ch in range(CHB):
        G = gpool.tile([P, KG, 2 * D], F16, tag="G")
        nc.gpsimd.dma_gather(
            out_ap=G[:],
            in_ap=table.ap()[:, :],
            idxs_ap=t_in["IDX"][:, ch * IDXW : (ch + 1) * IDXW],
            num_idxs=NI,
            num_idxs_reg=NI,
            elem_size=2 * D,
        )
        # kv += pos (broadcast over g), in place
        kv4 = G[:].rearrange("p (k g) e -> p k g e", k=K27, g=G2)
        nc.vector.tensor_tensor(
            out=kv4,
            in0=kv4,
            in1=posrep[:].unsqueeze(2).broadcast_to((P, K27, G2, 2 * D)),
            op=mybir.AluOpType.add,
        )
        # scores
        prod = bpool.tile([P, K27, G2, D], F16, tag="pv")
        qch = (
            qR[:, ch * G2 : (ch + 1) * G2, :]
            .unsqueeze(1)
            .broadcast_to((P, K27, G2, D))
        )
        nc.vector.tensor_tensor(out=prod[:], in0=kv4[:, :, :, 0:D], in1=qch, op=mybir.AluOpType.mult)
        scoresR = spool.tile([P, KG * H], F16, tag="scoresR")
        nc.vector.tensor_reduce(
            out=scoresR[:].rearrange("p (kg h) -> p kg h", h=H),
            in_=prod[:].rearrange("p k g (h e) -> p (k g) h e", h=H),
            axis=mybir.AxisListType.X,
            op=mybir.AluOpType.add,
        )
        scoresS = spool.tile([P, KG * H], F16, tag="scoresS")
        nc.vector.tensor_copy(scoresS[:], neg60k[:])
        nc.vector.copy_predicated(
            out=scoresS[:],
            mask=t_in["MASK"][:, ch * KG * H : (ch + 1) * KG * H],
            data=scoresR[:],
        )
        # softmax over k
        ghk = lambda t: t[:].rearrange("p (k g h) -> p g h k", k=K27, g=G2, h=H)
        mx = spool.tile([P, G2 * H], F32, tag="mx")
        nc.vector.tensor_reduce(
            out=mx[:].rearrange("p (g h) -> p g h", g=G2),
            in_=ghk(scoresS),
            axis=mybir.AxisListType.X,
            op=mybir.AluOpType.max,
        )
        esub = spool.tile([P, KG * H], F16, tag="esub")
        nc.vector.tensor_tensor(
            out=ghk(esub),
            in0=ghk(scoresS),
            in1=mx[:].rearrange("p (g h) -> p g h", g=G2).unsqueeze(3).broadcast_to((P, G2, H, K27)),
            op=mybir.AluOpType.subtract,
        )
        eexp = spool.tile([P, KG * H], F16, tag="eexp")
        nc.scalar.activation(eexp[:], esub[:], mybir.ActivationFunctionType.Exp)
        ssum = spool.tile([P, G2 * H], F32, tag="ssum")
        nc.vector.tensor_reduce(
            out=ssum[:].rearrange("p (g h) -> p g h", g=G2),
            in_=ghk(eexp),
            axis=mybir.AxisListType.X,
            op=mybir.AluOpType.add,
        )
        sinv = spool.tile([P, G2 * H], F32, tag="sinv")
        nc.vector.reciprocal(sinv[:], ssum[:])
        sinv16 = spool.tile([P, G2 * H], F16, tag="sinv16")
        nc.vector.tensor_copy(sinv16[:], sinv[:])
        probs = spool.tile([P, KG * H], F16, tag="probs")
        nc.vector.tensor_tensor(
            out=ghk(probs),
            in0=ghk(eexp),
            in1=sinv16[:].rearrange("p (g h) -> p g h", g=G2).unsqueeze(3).broadcast_to((P, G2, H, K27)),
            op=mybir.AluOpType.mult,
        )
        # attention-weighted V
        vprod = bpool.tile([P, K27, G2 * D], F16, tag="pv")
        pvw = probs[:].rearrange("p (k g h) -> p k g h", k=K27, g=G2)
        for gg in range(G2):
            nc.vector.tensor_tensor(
                out=vprod[:, :, gg * D : (gg + 1) * D].rearrange("p k (h e) -> p k h e", h=H),
                in0=kv4[:, :, gg, D : 2 * D].rearrange("p k (h e) -> p k h e", h=H),
                in1=pvw[:, :, gg, :].unsqueeze(3).broadcast_to((P, K27, H, DH)),
                op=mybir.AluOpType.mult,
            )
        n = K27
        while n > 1:
            c = -(-n // 2)
            m = n - c
            nc.vector.tensor_tensor(
                out=vprod[:, 0:m, :],
                in0=vprod[:, 0:m, :],
                in1=vprod[:, c : c + m, :],
                op=mybir.AluOpType.add,
            )
            n = c
        # per 128-point group: out projection + residual + layernorm
        for gg in range(G2):
            gglob = ch * G2 + gg
            psT = psSm.tile([P, D], F16, tag="psSm")
            nc.tensor.matmul(psT[:], vprod[:, 0, gg * D : (gg + 1) * D], ident16[:], is_transpose=True, start=True, stop=True)
            attnT = spool.tile([P, D], F16, tag="attnT")
            nc.vector.tensor_copy(attnT[:], psT[:])
            psO = psSm.tile([P, D], F32, tag="psSm")
            nc.tensor.matmul(psO[:], attnT[:], owT16[:], start=True, stop=False)
            nc.tensor.matmul(psO[:], t_in["ones1h"][:], outb16[:], start=False, stop=True)
            enh = spool.tile([P, D], F32, tag="enh")
            nc.vector.scalar_tensor_tensor(
                out=enh[:],
                in0=psO[:],
                scalar=t_in["HAS"][:, gglob : gglob + 1],
                in1=featR[:, gglob, :],
                op0=mybir.AluOpType.mult,
                op1=mybir.AluOpType.add,
            )
            mu = spool.tile([P, 1], F32, tag="mu")
            nc.vector.tensor_reduce(out=mu[:], in_=enh[:], axis=mybir.AxisListType.X, op=mybir.AluOpType.add)
            mus = spool.tile([P, 1], F32, tag="mus")
            nc.scalar.activation(mus[:], mu[:], mybir.ActivationFunctionType.Copy, bias=0.0, scale=float(1.0 / D))
            cent = spool.tile([P, D], F32, tag="cent")
            nc.vector.tensor_scalar_sub(cent[:], enh[:], mus[:, 0:1])
            sq = spool.tile([P, D], F32, tag="sq")
            var = spool.tile([P, 1], F32, tag="var")
            nc.vector.tensor_tensor_reduce(
                out=sq[:],
                in0=cent[:],
                in1=cent[:],
                scale=float(1.0 / D),
                scalar=0.0,
                op0=mybir.AluOpType.mult,
                op1=mybir.AluOpType.add,
                accum_out=var[:],
            )
            sd = spool.tile([P, 1], F32, tag="sd")
            nc.scalar.activation(sd[:], var[:], mybir.ActivationFunctionType.Sqrt, bias=t_in["eps_col"][:, 0:1])
            rinv = spool.tile([P, 1], F32, tag="rinv")
            nc.vector.reciprocal(rinv[:], sd[:])
            y1 = spool.tile([P, D], F32, tag="y1")
            nc.vector.tensor_scalar_mul(y1[:], cent[:], rinv[:, 0:1])
            y2 = spool.tile([P, D], F32, tag="y2")
            nc.vector.tensor_tensor(out=y2[:], in0=y1[:], in1=gammarep[:], op=mybir.AluOpType.mult)
            y3 = spool.tile([P, D], F32, tag="y3")
            nc.vector.tensor_tensor(out=y3[:], in0=y2[:], in1=betarep[:], op=mybir.AluOpType.add)
            nc.sync.dma_start(out_d.ap()[:, gglob, :], y3[:])


# ----------------------------------------------------------------- entry point
def build_in_maps(inputs, XS, S, per_core):
    in_w = np.asarray(inputs["in_w"], np.float32)
    in_b = np.asarray(inputs["in_b"], np.float32)
    rr = np.arange(-1, 2)
    offs = np.stack(np.meshgrid(rr, rr, rr, indexing="ij"), -1).reshape(-1, 3)

    onehot32 = np.zeros((1, DIN + 1), np.float32)
    onehot32[0, DIN] = 1.0
    rep = {
        "XS": XS,
        "S": S,
        "fW_aug": np.concatenate([inputs["fW"], inputs["fb"][None, :]], 0).astype(np.float32),
        "fW_augT": np.concatenate([inputs["fW"], inputs["fb"][None, :]], 0).T.astype(np.float32),
        "aW": np.asarray(inputs["aW"], np.float32),
        "ab_row": np.asarray(inputs["ab"], np.float32)[None, :],
        "WqT": in_w[:D].T.copy(),
        "WkT": in_w[D : 2 * D].T.copy(),
        "WvT": in_w[2 * D :].T.copy(),
        "out_wT": np.asarray(inputs["out_w"], np.float32).T.copy(),
        "bq_row": in_b[:D][None, :].copy(),
        "bk_col": in_b[D : 2 * D][:, None].copy(),
        "bv_col": in_b[2 * D :][:, None].copy(),
        "out_b_row": np.asarray(inputs["out_b"], np.float32)[None, :],
        "pW1": np.asarray(inputs["pW1"], np.float32),
        "pb1_col": np.asarray(inputs["pb1"], np.float32)[:, None],
        "pW2": np.asarray(inputs["pW2"], np.float32),
        "pb2_col": np.asarray(inputs["pb2"], np.float32)[:, None],
        "offs_fT": offs.T.astype(np.float32),
        "gamma_row": np.asarray(inputs["gamma"], np.float32)[None, :],
        "beta_row": np.asarray(inputs["beta"], np.float32)[None, :],
        "eps_col": np.full((D, 1), LN_EPS, np.float32),
        "onehot32": onehot32,
        "ones1f": np.ones((1, P), np.float32),
        "ones1h": np.ones((1, P), np.float16),
    }
    in_maps = []
    for core in range(NCORES):
        m = dict(rep)
        m["XO"] = per_core[core]["XO"]
        m["IDX"] = per_core[core]["IDX"]
        m["MASK"] = per_core[core]["MASK"]
        m["HAS"] = per_core[core]["HAS"]
        in_maps.append(m)
    return in_maps


def kernel(**inputs):
    features = np.asarray(inputs["features"], np.float32)
    coords = np.asarray(inputs["coords"], np.float32)

    XS, S, CA, R_TAB, per_core = _host_prep(features, coords)

    key = (CA, R_TAB)
    if key not in _cache:
        _cache[key] = _build(CA, R_TAB)
    nc = _cache[key]

    in_maps = build_in_maps(inputs, XS, S, per_core)
    res = bass_utils.run_bass_kernel_spmd(nc, in_maps, core_ids=list(range(NCORES)))

    out = np.zeros((B * N, D), np.float32)
    for core in range(NCORES):
        o = res.results[core]["OUT"]  # (P, CHB*G2, D)
        for gg in range(CHB * G2):
            out[core * PTS_CORE + gg * P : core * PTS_CORE + (gg + 1) * P] = o[:, gg, :]
    return out.reshape(B, N, D)


# revision 16
# speedup vs baseline: 1.1966x; 1.1966x over previous
"""MicroVoxelSpatialEncoder Trainium2 kernel.

Strategy (8 NeuronCores, no collectives):
  - Host (numpy, integer/index work only): voxel hashing, per-batch sort of
    points by voxel id, segment/selection matrices, neighbor gather indices,
    validity masks.
  - Device phase A (replicated on every core): build a per-voxel table of
    pre-projected K/V rows (fp16, [K|V] = 512B per voxel) in DRAM.
    Segment means are computed with PE selection matmuls (1/cnt folded into
    the selection matrix), the intra-voxel MLP + K/V projections are PE
    matmuls, positional-encoding MLP included.
  - Device phase B (sharded: core c owns flat points [c*1024,(c+1)*1024)):
    dma_gather 27 neighbor rows per point, masked per-point multi-head
    attention, output projection, residual + layernorm.
"""

import os
import sys

import numpy as np

for _p in ("/opt/trn_rl_repo", "/root/.axon_site/_ro/trn_rl_repo"):
    if _p not in sys.path and os.path.isdir(_p):
        sys.path.append(_p)

import concourse.bacc as bacc
import concourse.bass as bass
import concourse.tile as tile
from concourse import bass_utils, mybir
from concourse.masks import make_identity

F32 = mybir.dt.float32
F32R = mybir.dt.float32r
F16 = mybir.dt.float16
I16 = mybir.dt.int16

GRID = (64, 64, 100)
V_GRID = GRID[0] * GRID[1] * GRID[2]
MINS = np.array([0.0, 0.0, 0.0], np.float32)
MAXS = np.array([256.0, 256.0, 1.0], np.float32)
H = 4
LN_EPS = 1e-5

P = 128
B, N, DIN, D = 2, 4096, 32, 128
DH = D // H
K27 = 27
NCORES = 8
PTS_CORE = (B * N) // NCORES      # 1024
CHB = 4                            # phase-B chunks per core
PTS_CH = PTS_CORE // CHB           # 256
G2 = PTS_CH // P                   # 2
KG = K27 * G2                      # 54
NI = KG * P                        # 6912 gather rows per chunk
IDXW = NI // 16                    # 432

_cache = {}


# ----------------------------------------------------------------- host prep
def _voxelize(coords):
    """Exactly mirror the reference's fp32 voxel hashing."""
    c = coords.astype(np.float32)
    norm = np.clip((c - MINS) / (MAXS - MINS), np.float32(0.0), np.float32(1.0))
    g = np.array(GRID, np.int32)
    vidx = (norm * (g - 1).astype(np.float32)).astype(np.int32)
    vid = (vidx[..., 0] * g[1] + vidx[..., 1]) * g[2] + vidx[..., 2]
    return vidx, vid


def _host_prep(features, coords):
    vidx, vid = _voxelize(coords)

    # --- per-batch sort by voxel id; chunk so no voxel-run spans a chunk ---
    slot_maps = []      # per batch: dict vid -> slot
    chunks = []         # list of (batch, [(sorted_point_indices, r)] per chunk)
    for b in range(B):
        order = np.argsort(vid[b], kind="stable")
        svid = vid[b][order]
        uniq, starts, cnts = np.unique(svid, return_index=True, return_counts=True)
        if cnts.max() > P:
            raise NotImplementedError("voxel with more than 128 points")
        runs = list(zip(uniq.tolist(), starts.tolist(), cnts.tolist()))
        cur = []  # list of runs in current chunk
        cur_n = 0
        bchunks = []
        for u, s, c in runs:
            if cur_n + c > P:
                bchunks.append(cur)
                cur, cur_n = [], 0
            cur.append((u, s, c))
            cur_n += c
        if cur:
            bchunks.append(cur)
        chunks.append((b, bchunks, order))
        slot_maps.append({})

    ca_list = [len(chunks[b][1]) for b in range(B)]
    CA = sum(ca_list)
    CA = -(-CA // 4) * 4  # pad total chunk count to a multiple of 4
    # distribute padding chunks (all-dummy) at the end
    R_TAB = CA * P + 16   # table rows: slots + dummy row at CA*P (rest pad)
    DUMMY = CA * P

    feats_flat = features.reshape(B * N, DIN).astype(np.float32)

    XS = np.zeros((P, CA, DIN + 1), np.float32)
    S = np.zeros((P, CA * P), np.float16)
    c_global = 0
    for b, bchunks, order in chunks:
        for runs in bchunks:
            p0 = 0
            for j, (u, s, c) in enumerate(runs):
                idxs = order[s : s + c]  # original point indices in batch b
                r = np.float16(1.0 / np.float32(c))
                for pi in idxs:
                    XS[p0, c_global, :DIN] = feats_flat[b * N + pi]
                    XS[p0, c_global, DIN] = 1.0
                    S[p0, c_global * P + j] = r
                    p0 += 1
                slot_maps[b][u] = c_global * P + j
            c_global += 1
    XS = XS.reshape(P, CA * (DIN + 1))

    # --- neighbor indices / masks per core ---
    rr = np.arange(-1, 2)
    offs = np.stack(np.meshgrid(rr, rr, rr, indexing="ij"), -1).reshape(-1, 3)
    g = np.array(GRID, np.int32)

    vidx_flat = vidx.reshape(B * N, 3)
    nidx = vidx_flat[:, None, :] + offs[None, :, :]          # (BN, 27, 3)
    inb = np.all((nidx >= 0) & (nidx < g), axis=-1)          # (BN, 27)
    ncl = np.clip(nidx, 0, g - 1)
    nvid = (ncl[..., 0] * g[1] + ncl[..., 1]) * g[2] + ncl[..., 2]

    j_all = np.full((B * N, K27), DUMMY, np.int64)
    valid = np.zeros((B * N, K27), bool)
    for b in range(B):
        sm = slot_maps[b]
        base = b * N
        for i in range(N):
            row = nvid[base + i]
            ib = inb[base + i]
            for k in range(K27):
                if ib[k]:
                    s = sm.get(int(row[k]))
                    if s is not None:
                        j_all[base + i, k] = s
                        valid[base + i, k] = True
    has = valid.any(-1)

    per_core = []
    for core in range(NCORES):
        sl = slice(core * PTS_CORE, (core + 1) * PTS_CORE)
        jc = j_all[sl].reshape(CHB, G2, P, K27)
        # gather order i = (k*G2+g)*P + p  for each chunk
        idx = np.zeros((CHB, K27, G2, P), np.int64)
        for ch in range(CHB):
            for k in range(K27):
                for gg in range(G2):
                    idx[ch, k, gg] = jc[ch, gg, :, k]
        idx = idx.reshape(CHB, NI)
        blocks = []
        for ch in range(CHB):
            blk = np.zeros((16, IDXW), np.int16)
            lin = idx[ch]
            blk[np.arange(NI) % 16, np.arange(NI) // 16] = lin.astype(np.int16)
            blocks.append(np.tile(blk, (8, 1)))
        IDX = np.concatenate(blocks, axis=1)  # (128, CHB*IDXW)

        vc = valid[sl].reshape(CHB, G2, P, K27)
        MASK = np.zeros((P, CHB * KG * H), np.float16)
        for ch in range(CHB):
            for k in range(K27):
                for gg in range(G2):
                    for hh in range(H):
                        MASK[:, (ch * KG + k * G2 + gg) * H + hh] = vc[ch, gg, :, k]
        HAS = np.zeros((P, CHB * G2), np.float32)
        hc = has[sl].reshape(CHB * G2, P)
        for gg in range(CHB * G2):
            HAS[:, gg] = hc[gg]
        XO = np.zeros((DIN + 1, PTS_CORE), np.float32)
        XO[:DIN] = feats_flat[sl].T
        XO[DIN] = 1.0
        per_core.append({"IDX": IDX, "MASK": MASK, "HAS": HAS, "XO": XO})

    return XS, S, CA, R_TAB, per_core


# ------------------------------------------------------------- device program
def _build(CA, R_TAB):
    nc = bacc.Bacc(
        "TRN2",
        target_bir_lowering=False,
        debug=False,
        enable_asserts=False,
        num_devices=1,
    )
    CAP = CA * P
    dt_in = {
        "XS": ([P, CA * (DIN + 1)], F32),
        "S": ([P, CAP], F16),
        "XO": ([DIN + 1, PTS_CORE], F32),
        "IDX": ([P, CHB * IDXW], I16),
        "MASK": ([P, CHB * KG * H], F16),
        "HAS": ([P, CHB * G2], F32),
        "fW_aug": ([DIN + 1, D], F32),
        "fW_augT": ([D, DIN + 1], F32),
        "aW": ([D, D], F32),
        "ab_row": ([1, D], F32),
        "WqT": ([D, D], F32),
        "WkT": ([D, D], F32),
        "WvT": ([D, D], F32),
        "out_wT": ([D, D], F32),
        "bq_row": ([1, D], F32),
        "bk_col": ([D, 1], F32),
        "bv_col": ([D, 1], F32),
        "out_b_row": ([1, D], F32),
        "pW1": ([3, D // 2], F32),
        "pb1_col": ([D // 2, 1], F32),
        "pW2": ([D // 2, D], F32),
        "pb2_col": ([D, 1], F32),
        "offs_fT": ([3, K27], F32),
        "gamma_row": ([1, D], F32),
        "beta_row": ([1, D], F32),
        "eps_col": ([D, 1], F32),
        "onehot32": ([1, DIN + 1], F32),
        "ones1f": ([1, P], F32),
        "ones1h": ([1, P], F16),
        "headmask": ([D, H], F16),
    }
    dram = {k: nc.dram_tensor(k, shp, dt, kind="ExternalInput") for k, (shp, dt) in dt_in.items()}
    out_d = nc.dram_tensor("OUT", [P, CHB * G2, D], F32, kind="ExternalOutput")
    table = nc.dram_tensor("tableKV", [R_TAB, 2 * D], F16, kind="Internal")

    from contextlib import ExitStack

    with tile.TileContext(nc) as tc:
        with ExitStack() as ctx, nc.allow_low_precision(
            "fp16 phase-B math; tolerance-checked vs fp32 ref"
        ):
            _emit(ctx, tc, nc, dram, out_d, table, CA)
    nc.compile()
    return nc


def _emit(ctx, tc, nc, dram, out_d, table, CA):
    CAP = CA * P
    W512 = CAP // 512

    const = ctx.enter_context(tc.tile_pool(name="const", bufs=1))

    # ---------- load persistent inputs ----------
    t_in = {}
    for k in dram:
        if k in ("XS", "S"):
            continue
        shp = list(dram[k].shape)
        tl = const.tile(shp, dram[k].dtype, tag=f"in_{k}")
        nc.sync.dma_start(tl[:], dram[k].ap())
        t_in[k] = tl

    ident16 = const.tile([P, P], F16, tag="ident16")
    make_identity(nc, ident16[:])
    neg60k = const.tile([P, KG * H], F16, tag="neg60k")
    nc.vector.memset(neg60k[:], -60000.0)

    # fp16 casts of weights (persistent)
    WkT16 = const.tile([D, D], F16, tag="WkT16")
    nc.vector.tensor_copy(WkT16[:], t_in["WkT"][:])
    WvT16 = const.tile([D, D], F16, tag="WvT16")
    nc.vector.tensor_copy(WvT16[:], t_in["WvT"][:])
    owT16 = const.tile([D, D], F16, tag="owT16")
    nc.vector.tensor_copy(owT16[:], t_in["out_wT"][:])
    outb16 = const.tile([1, D], F16, tag="outb16")
    nc.vector.tensor_copy(outb16[:], t_in["out_b_row"][:])

    # persistent phase-A outputs used by phase B
    featR = const.tile([P, CHB * G2, D], F32, tag="featR")
    qR = const.tile([P, CHB * G2, D], F16, tag="qR")
    QPr = const.tile([P, CHB * G2, K27 * H], F16, tag="QPr")
    posVBr = const.tile([K27 * H, D], F16, tag="posVBr")
    gammarep = const.tile([P, D], F32, tag="gammarep")
    betarep = const.tile([P, D], F32, tag="betarep")

    psSm = ctx.enter_context(tc.tile_pool(name="psSm", bufs=2, space="PSUM"))
    psB = ctx.enter_context(tc.tile_pool(name="psB", bufs=2, space="PSUM"))

    # ---------- phase A ----------
    with tc.tile_pool(name="pha", bufs=1) as pha, \
         tc.tile_pool(name="psA", bufs=2, space="PSUM") as psA, \
         tc.tile_pool(name="psW", bufs=2, space="PSUM") as psW, \
         tc.tile_pool(name="stage", bufs=2) as stage:
        XS32 = pha.tile([P, CA * (DIN + 1)], F32, tag="XS32")
        nc.sync.dma_start(XS32[:], dram["XS"].ap())
        S16 = pha.tile([P, CAP], F16, tag="S16")
        nc.sync.dma_start(S16[:], dram["S"].ap())
        XS16 = pha.tile([P, CA * (DIN + 1)], F16, tag="XS16")
        nc.vector.tensor_copy(XS16[:], XS32[:])

        # W2_aug = fW_aug @ aW (+ ab row); Wq2_aug = fW_aug @ Wq.T (+ bq row)
        W2s16 = pha.tile([DIN + 1, D], F16, tag="W2s16")
        Wq2s = pha.tile([DIN + 1, D], F32, tag="Wq2s")
        for dst, rhs_w, brow in ((W2s16, "aW", "ab_row"), (Wq2s, "WqT", "bq_row")):
            ps = psSm.tile([DIN + 1, D], F32, tag="psSm")
            nc.tensor.matmul(ps[:], t_in["fW_augT"][:], t_in[rhs_w][:], start=True, stop=False)
            nc.tensor.matmul(ps[:], t_in["onehot32"][:], t_in[brow][:], start=False, stop=True)
            nc.vector.tensor_copy(dst[:], ps[:])

        # segment sums (means: 1/cnt folded into S): gsumA[a, u], drains 4-wide
        gsumA = pha.tile([DIN + 1, CAP], F16, tag="gsumA")
        for c4 in range(CA // 4):
            ps = psA.tile([DIN + 1, 512], F32, tag="psA")
            for cc in range(4):
                c = c4 * 4 + cc
                lhs = XS16[:, c * (DIN + 1) : (c + 1) * (DIN + 1)]
                nc.tensor.matmul(
                    ps[:, cc * P : (cc + 1) * P], lhs, S16[:, c * P : (c + 1) * P], start=True, stop=True
                )
            if c4 % 2 == 0:
                nc.vector.tensor_copy(gsumA[:, c4 * 512 : (c4 + 1) * 512], ps[:])
            else:
                nc.scalar.copy(gsumA[:, c4 * 512 : (c4 + 1) * 512], ps[:])

        # grid MLP: gridT16[do, u] = relu(W2_aug.T @ gsumA)
        gridT16 = pha.tile([D, CAP], F16, tag="gridT16")
        for w in range(W512):
            ps = psW.tile([D, 512], F32, tag="psW")
            nc.tensor.matmul(
                ps[:], W2s16[:], gsumA[:, w * 512 : (w + 1) * 512], start=True, stop=True
            )
            nc.scalar.activation(gridT16[:, w * 512 : (w + 1) * 512], ps[:], mybir.ActivationFunctionType.Relu)

        # K/V tables -> DRAM (fused rows [K|V], fp16), 2 chunks per PSUM bank,
        # 4 chunks per staging tile / DMA
        for c4 in range(CA // 4):
            st = stage.tile([P, 4, 2 * D], F16, tag="stage")
            for half in range(2):
                ps = psW.tile([D, 512], F32, tag="psW")
                for q in range(2):
                    c = c4 * 4 + half * 2 + q
                    lhs = gridT16[:, c * P : (c + 1) * P]
                    nc.tensor.matmul(ps[:, q * 256 : q * 256 + D], lhs, WkT16[:], start=True, stop=True)
                    nc.tensor.matmul(ps[:, q * 256 + D : q * 256 + 2 * D], lhs, WvT16[:], start=True, stop=True)
                if half == 0:
                    nc.vector.tensor_copy(st[:, 0:2, :], ps[:].rearrange("p (c e) -> p c e", c=2))
                else:
                    nc.scalar.copy(st[:, 2:4, :], ps[:].rearrange("p (c e) -> p c e", c=2))
            dst = table.ap()[c4 * 4 * P : (c4 * 4 + 4) * P, :]
            dst = dst.rearrange("(c p) e -> p c e", p=P, c=4)
            nc.sync.dma_start(dst, st[:])
        zrow = stage.tile([16, 2 * D], F16, tag="zrow")
        nc.vector.memset(zrow[:], 0.0)
        nc.sync.dma_start(table.ap()[CAP : CAP + 16, :], zrow[:])

        # positional encodings -> posK/posV (k/v input biases folded in)
        ps = psSm.tile([D // 2, K27], F32, tag="psSm")
        nc.tensor.matmul(ps[:], t_in["pW1"][:], t_in["offs_fT"][:], start=True, stop=True)
        h1 = pha.tile([D // 2, K27], F32, tag="h1")
        nc.scalar.activation(h1[:], ps[:], mybir.ActivationFunctionType.Relu, bias=t_in["pb1_col"][:])
        ps = psSm.tile([D, K27], F32, tag="psSm")
        nc.tensor.matmul(ps[:], t_in["pW2"][:], h1[:], start=True, stop=True)
        posT = pha.tile([D, K27], F32, tag="posT")
        nc.scalar.activation(posT[:], ps[:], mybir.ActivationFunctionType.Identity, bias=t_in["pb2_col"][:])

        posKV16 = pha.tile([D, 2, K27], F16, tag="posKV16")
        for i, (wname, bcol) in enumerate((("WkT", "bk_col"), ("WvT", "bv_col"))):
            ps = psSm.tile([D, K27], F32, tag="psSm")
            nc.tensor.matmul(ps[:], t_in[wname][:], posT[:], start=True, stop=True)
            nc.scalar.activation(posKV16[:, i, :], ps[:], mybir.ActivationFunctionType.Identity, bias=t_in[bcol][:])

        # head-blocked positional tables:
        #   posKB[d, k*H+h] = posK[d, k] * (d in head h)   (for q . posK on PE)
        #   posVBr[(k h), e] = posV[e, k] * (e in head h)  (for sum_k eexp*posV on PE)
        posKB = pha.tile([D, K27 * H], F16, tag="posKB")
        posVB = pha.tile([D, K27 * H], F16, tag="posVB")
        for i, dst in enumerate((posKB, posVB)):
            nc.vector.tensor_tensor(
                out=dst[:].rearrange("p (k h) -> p k h", k=K27),
                in0=posKV16[:, i, :].unsqueeze(2).broadcast_to((D, K27, H)),
                in1=t_in["headmask"][:].unsqueeze(1).broadcast_to((D, K27, H)),
                op=mybir.AluOpType.mult,
            )
        ps = psSm.tile([K27 * H, D], F16, tag="psSm")
        nc.tensor.matmul(ps[:], posVB[:], ident16[:], is_transpose=True, start=True, stop=True)
        nc.vector.tensor_copy(posVBr[:], ps[:])

        # qT16 (for QP): qT[dq, i] = Wq2_aug.T @ XO, scaled by 1/sqrt(dh)
        qT16 = pha.tile([D, PTS_CORE], F16, tag="qT16")
        for w in range(PTS_CORE // 512):
            ps = psW.tile([D, 512], F32, tag="psW")
            nc.tensor.matmul(ps[:], Wq2s[:], t_in["XO"][:, w * 512 : (w + 1) * 512], start=True, stop=True)
            nc.scalar.activation(
                qT16[:, w * 512 : (w + 1) * 512], ps[:], mybir.ActivationFunctionType.Copy,
                bias=0.0, scale=float(1.0 / np.sqrt(DH)),
            )
        # QP[i, (k h)] = sum_d qT16[d, i] * posKB[d, (k h)] -> transpose per group
        QPT = pha.tile([K27 * H, PTS_CORE], F16, tag="QPT")
        for w in range(PTS_CORE // 512):
            ps = psW.tile([K27 * H, 512], F32, tag="psW")
            nc.tensor.matmul(ps[:], posKB[:], qT16[:, w * 512 : (w + 1) * 512], start=True, stop=True)
            nc.scalar.copy(QPT[:, w * 512 : (w + 1) * 512], ps[:])
        for gg in range(CHB * G2):
            ps = psSm.tile([P, K27 * H], F16, tag="psSm")
            nc.tensor.matmul(
                ps[:], QPT[:, gg * P : (gg + 1) * P], ident16[0 : K27 * H, 0 : K27 * H],
                is_transpose=True, start=True, stop=True,
            )
            nc.vector.tensor_copy(QPr[:, gg, :], ps[:])

        # feat / q rows for this core's points (2 groups per PSUM bank)
        for pair in range(CHB * G2 // 2):
            psf = psB.tile([P, 2 * D], F32, tag="psB")
            psq = psB.tile([P, 2 * D], F32, tag="psB")
            for q in range(2):
                gg = pair * 2 + q
                xo = t_in["XO"][:, gg * P : (gg + 1) * P]
                nc.tensor.matmul(psf[:, q * D : (q + 1) * D], xo, t_in["fW_aug"][:], start=True, stop=True)
                nc.tensor.matmul(psq[:, q * D : (q + 1) * D], xo, Wq2s[:], start=True, stop=True)
            nc.vector.tensor_copy(featR[:, pair * 2 : pair * 2 + 2, :], psf[:].rearrange("p (g e) -> p g e", g=2))
            nc.scalar.activation(
                qR[:, pair * 2 : pair * 2 + 2, :].rearrange("p g e -> p (g e)"),
                psq[:],
                mybir.ActivationFunctionType.Copy,
                bias=0.0,
                scale=float(1.0 / np.sqrt(DH)),
            )

        for dst, row in ((gammarep, "gamma_row"), (betarep, "beta_row")):
            ps = psSm.tile([P, D], F32, tag="psSm")
            nc.tensor.matmul(ps[:], t_in["ones1f"][:], t_in[row][:], start=True, stop=True)
            nc.vector.tensor_copy(dst[:], ps[:])

    # ---------- phase B ----------
    gpool = ctx.enter_context(tc.tile_pool(name="gpool", bufs=2))
    bpool = ctx.enter_context(tc.tile_pool(name="bpool", bufs=2))
    spool = ctx.enter_context(tc.tile_pool(name="spool", bufs=2))

    for ch in range(CHB):
        G = gpool.tile([P, KG, 2 * D], F16, tag="G")
        nc.gpsimd.dma_gather(
            out_ap=G[:],
            in_ap=table.ap()[:, :],
            idxs_ap=t_in["IDX"][:, ch * IDXW : (ch + 1) * IDXW],
            num_idxs=NI,
            num_idxs_reg=NI,
            elem_size=2 * D,
        )
        kv4 = G[:].rearrange("p (k g) e -> p k g e", k=K27, g=G2)
        # scores from gathered K: prod then tree-reduce over d (within heads)
        prod = bpool.tile([P, K27, G2, D], F16, tag="pv")
        qch = (
            qR[:, ch * G2 : (ch + 1) * G2, :]
            .unsqueeze(1)
            .broadcast_to((P, K27, G2, D))
        )
        nc.vector.tensor_tensor(out=prod[:], in0=kv4[:, :, :, 0:D], in1=qch, op=mybir.AluOpType.mult)
        w = DH // 2
        while w >= 1:
            pr4 = prod[:].rearrange("p k g (h e) -> p (k g) h e", h=H)
            nc.vector.tensor_tensor(
                out=pr4[:, :, :, 0:w],
                in0=pr4[:, :, :, 0:w],
                in1=pr4[:, :, :, w : 2 * w],
                op=mybir.AluOpType.add,
            )
            w //= 2
        # scores = tree result + q.posK term
        scoresR = spool.tile([P, KG * H], F16, tag="scoresR")
        nc.vector.tensor_tensor(
            out=scoresR[:].rearrange("p (k g h) -> p k g h", k=K27, g=G2),
            in0=prod[:, :, :, :].rearrange("p k g (h e) -> p k g h e", h=H)[:, :, :, :, 0],
            in1=QPr[:, ch * G2 : (ch + 1) * G2, :]
            .rearrange("p g (k h) -> p k g h", k=K27),
            op=mybir.AluOpType.add,
        )
        scoresS = spool.tile([P, KG * H], F16, tag="scoresS")
        nc.vector.tensor_copy(scoresS[:], neg60k[:])
        nc.vector.copy_predicated(
            out=scoresS[:],
            mask=t_in["MASK"][:, ch * KG * H : (ch + 1) * KG * H],
            data=scoresR[:],
        )
        # softmax over k (unnormalized; 1/sum applied at the end)
        ghk = lambda t: t[:].rearrange("p (k g h) -> p g h k", k=K27, g=G2, h=H)
        mx = spool.tile([P, G2 * H], F16, tag="mx")
        nc.vector.tensor_reduce(
            out=mx[:].rearrange("p (g h) -> p g h", g=G2),
            in_=ghk(scoresS),
            axis=mybir.AxisListType.X,
            op=mybir.AluOpType.max,
        )
        esub = spool.tile([P, KG * H], F16, tag="esub")
        nc.vector.tensor_tensor(
            out=ghk(esub),
            in0=ghk(scoresS),
            in1=mx[:].rearrange("p (g h) -> p g h", g=G2).unsqueeze(3).broadcast_to((P, G2, H, K27)),
            op=mybir.AluOpType.subtract,
        )
        eexp = spool.tile([P, KG * H], F16, tag="eexp")
        nc.scalar.activation(eexp[:], esub[:], mybir.ActivationFunctionType.Exp)
        ssum = spool.tile([P, G2 * H], F32, tag="ssum")
        nc.vector.tensor_reduce(
            out=ssum[:].rearrange("p (g h) -> p g h", g=G2),
            in_=ghk(eexp),
            axis=mybir.AxisListType.X,
            op=mybir.AluOpType.add,
        )
        sinv = spool.tile([P, G2 * H], F32, tag="sinv")
        nc.vector.reciprocal(sinv[:], ssum[:])
        sinv16 = spool.tile([P, G2 * H], F16, tag="sinv16")
        nc.vector.tensor_copy(sinv16[:], sinv[:])
        # eexp-weighted V, tree-summed over k
        vprod = bpool.tile([P, K27, G2 * D], F16, tag="pv")
        ew = eexp[:].rearrange("p (k g h) -> p k g h", k=K27, g=G2)
        for gg in range(G2):
            nc.vector.tensor_tensor(
                out=vprod[:, :, gg * D : (gg + 1) * D].rearrange("p k (h e) -> p k h e", h=H),
                in0=kv4[:, :, gg, D : 2 * D].rearrange("p k (h e) -> p k h e", h=H),
                in1=ew[:, :, gg, :].unsqueeze(3).broadcast_to((P, K27, H, DH)),
                op=mybir.AluOpType.mult,
            )
        n = K27
        while n > 1:
            c = -(-n // 2)
            m = n - c
            nc.vector.tensor_tensor(
                out=vprod[:, 0:m, :],
                in0=vprod[:, 0:m, :],
                in1=vprod[:, c : c + m, :],
                op=mybir.AluOpType.add,
            )
            n = c
        # per-chunk epilogue
        enh = spool.tile([P, G2, D], F32, tag="enh")
        for gg in range(G2):
            gglob = ch * G2 + gg
            # eexp^T for this group (PE) -> PV = sum_k eexp*posV (PE)
            psE = psSm.tile([K27 * H, P], F16, tag="psSm")
            nc.tensor.matmul(
                psE[:],
                eexp[:].rearrange("p (k g h) -> p k g h", k=K27, g=G2)[:, :, gg, :],
                ident16[:],
                is_transpose=True,
                start=True,
                stop=True,
            )
            eexpT = spool.tile([K27 * H, P], F16, tag="eexpT")
            nc.vector.tensor_copy(eexpT[:], psE[:])
            psPV = psB.tile([P, D], F32, tag="psB")
            nc.tensor.matmul(psPV[:], eexpT[:], posVBr[:], start=True, stop=True)
            # attn_unnorm = vprod_g + PV ; attn = attn_unnorm * sinv (per head)
            att = spool.tile([P, D], F16, tag="att")
            nc.vector.tensor_tensor(
                out=att[:], in0=vprod[:, 0, gg * D : (gg + 1) * D], in1=psPV[:], op=mybir.AluOpType.add
            )
            attn = spool.tile([P, D], F16, tag="attn")
            nc.vector.tensor_tensor(
                out=attn[:].rearrange("p (h e) -> p h e", h=H),
                in0=att[:].rearrange("p (h e) -> p h e", h=H),
                in1=sinv16[:, gg * H : (gg + 1) * H].unsqueeze(2).broadcast_to((P, H, DH)),
                op=mybir.AluOpType.mult,
            )
            psT = psSm.tile([P, D], F16, tag="psSm")
            nc.tensor.matmul(psT[:], attn[:], ident16[:], is_transpose=True, start=True, stop=True)
            attnT = spool.tile([P, D], F16, tag="attnT")
            nc.vector.tensor_copy(attnT[:], psT[:])
            psO = psB.tile([P, D], F32, tag="psB")
            nc.tensor.matmul(psO[:], attnT[:], owT16[:], start=True, stop=False)
            nc.tensor.matmul(psO[:], t_in["ones1h"][:], outb16[:], start=False, stop=True)
            nc.vector.scalar_tensor_tensor(
                out=enh[:, gg, :],
                in0=psO[:],
                scalar=t_in["HAS"][:, gglob : gglob + 1],
                in1=featR[:, gglob, :],
                op0=mybir.AluOpType.mult,
                op1=mybir.AluOpType.add,
            )
        # layernorm, both groups at once
        mu = spool.tile([P, G2], F32, tag="mu")
        nc.vector.tensor_reduce(out=mu[:], in_=enh[:], axis=mybir.AxisListType.X, op=mybir.AluOpType.add)
        mus = spool.tile([P, G2], F32, tag="mus")
        nc.vector.tensor_scalar_mul(mus[:], mu[:], float(1.0 / D))
        cent = spool.tile([P, G2, D], F32, tag="cent")
        nc.vector.tensor_tensor(
            out=cent[:], in0=enh[:], in1=mus[:].unsqueeze(2).broadcast_to((P, G2, D)), op=mybir.AluOpType.subtract
        )
        sq = spool.tile([P, G2, D], F32, tag="sq")
        nc.vector.tensor_tensor(out=sq[:], in0=cent[:], in1=cent[:], op=mybir.AluOpType.mult)
        var = spool.tile([P, G2], F32, tag="var")
        nc.vector.tensor_reduce(out=var[:], in_=sq[:], axis=mybir.AxisListType.X, op=mybir.AluOpType.add)
        varm = spool.tile([P, G2], F32, tag="varm")
        nc.vector.tensor_scalar_mul(varm[:], var[:], float(1.0 / D))
        sd = spool.tile([P, G2], F32, tag="sd")
        nc.scalar.activation(sd[:], varm[:], mybir.ActivationFunctionType.Sqrt, bias=t_in["eps_col"][:, 0:1])
        rinv = spool.tile([P, G2], F32, tag="rinv")
        nc.vector.reciprocal(rinv[:], sd[:])
        y1 = spool.tile([P, G2, D], F32, tag="y1")
        nc.vector.tensor_tensor(
            out=y1[:], in0=cent[:], in1=rinv[:].unsqueeze(2).broadcast_to((P, G2, D)), op=mybir.AluOpType.mult
        )
        y2 = spool.tile([P, G2, D], F32, tag="y2")
        nc.vector.tensor_tensor(
            out=y2[:], in0=y1[:], in1=gammarep[:].unsqueeze(1).broadcast_to((P, G2, D)), op=mybir.AluOpType.mult
        )
        y3 = spool.tile([P, G2, D], F32, tag="y3")
        nc.vector.tensor_tensor(
            out=y3[:], in0=y2[:], in1=betarep[:].unsqueeze(1).broadcast_to((P, G2, D)), op=mybir.AluOpType.add
        )
        nc.sync.dma_start(out_d.ap()[:, ch * G2 : (ch + 1) * G2, :], y3[:])


# ----------------------------------------------------------------- entry point
def build_in_maps(inputs, XS, S, per_core):
    in_w = np.asarray(inputs["in_w"], np.float32)
    in_b = np.asarray(inputs["in_b"], np.float32)
    rr = np.arange(-1, 2)
    offs = np.stack(np.meshgrid(rr, rr, rr, indexing="ij"), -1).reshape(-1, 3)

    onehot32 = np.zeros((1, DIN + 1), np.float32)
    onehot32[0, DIN] = 1.0
    rep = {
        "XS": XS,
        "S": S,
        "fW_aug": np.concatenate([inputs["fW"], inputs["fb"][None, :]], 0).astype(np.float32),
        "fW_augT": np.concatenate([inputs["fW"], inputs["fb"][None, :]], 0).T.astype(np.float32),
        "aW": np.asarray(inputs["aW"], np.float32),
        "ab_row": np.asarray(inputs["ab"], np.float32)[None, :],
        "WqT": in_w[:D].T.copy(),
        "WkT": in_w[D : 2 * D].T.copy(),
        "WvT": in_w[2 * D :].T.copy(),
        "out_wT": np.asarray(inputs["out_w"], np.float32).T.copy(),
        "bq_row": in_b[:D][None, :].copy(),
        "bk_col": in_b[D : 2 * D][:, None].copy(),
        "bv_col": in_b[2 * D :][:, None].copy(),
        "out_b_row": np.asarray(inputs["out_b"], np.float32)[None, :],
        "pW1": np.asarray(inputs["pW1"], np.float32),
        "pb1_col": np.asarray(inputs["pb1"], np.float32)[:, None],
        "pW2": np.asarray(inputs["pW2"], np.float32),
        "pb2_col": np.asarray(inputs["pb2"], np.float32)[:, None],
        "offs_fT": offs.T.astype(np.float32),
        "gamma_row": np.asarray(inputs["gamma"], np.float32)[None, :],
        "beta_row": np.asarray(inputs["beta"], np.float32)[None, :],
        "eps_col": np.full((D, 1), LN_EPS, np.float32),
        "onehot32": onehot32,
        "ones1f": np.ones((1, P), np.float32),
        "ones1h": np.ones((1, P), np.float16),
        "headmask": (np.arange(D)[:, None] // DH == np.arange(H)[None, :]).astype(np.float16),
    }
    in_maps = []
    for core in range(NCORES):
        m = dict(rep)
        m["XO"] = per_core[core]["XO"]
        m["IDX"] = per_core[core]["IDX"]
        m["MASK"] = per_core[core]["MASK"]
        m["HAS"] = per_core[core]["HAS"]
        in_maps.append(m)
    return in_maps


def kernel(**inputs):
    features = np.asarray(inputs["features"], np.float32)
    coords = np.asarray(inputs["coords"], np.float32)

    XS, S, CA, R_TAB, per_core = _host_prep(features, coords)

    key = (CA, R_TAB)
    if key not in _cache:
        _cache[key] = _build(CA, R_TAB)
    nc = _cache[key]

    in_maps = build_in_maps(inputs, XS, S, per_core)
    res = bass_utils.run_bass_kernel_spmd(nc, in_maps, core_ids=list(range(NCORES)))

    out = np.zeros((B * N, D), np.float32)
    for core in range(NCORES):
        o = res.results[core]["OUT"]  # (P, CHB*G2, D)
        for gg in range(CHB * G2):
            out[core * PTS_CORE + gg * P : core * PTS_CORE + (gg + 1) * P] = o[:, gg, :]
    return out.reshape(B, N, D)


# revision 18
# speedup vs baseline: 1.2795x; 1.0693x over previous
"""MicroVoxelSpatialEncoder Trainium2 kernel.

Strategy (8 NeuronCores, no collectives):
  - Host (numpy, integer/index work only): voxel hashing, per-batch sort of
    points by voxel id, segment/selection matrices, neighbor gather indices,
    validity masks.
  - Device phase A (replicated on every core): build a per-voxel table of
    pre-projected K/V rows (fp16, [K|V] = 512B per voxel) in DRAM.
    Segment means are computed with PE selection matmuls (1/cnt folded into
    the selection matrix), the intra-voxel MLP + K/V projections are PE
    matmuls, positional-encoding MLP included.
  - Device phase B (sharded: core c owns flat points [c*1024,(c+1)*1024)):
    dma_gather 27 neighbor rows per point, masked per-point multi-head
    attention, output projection, residual + layernorm.
"""

import os
import sys

import numpy as np

for _p in ("/opt/trn_rl_repo", "/root/.axon_site/_ro/trn_rl_repo"):
    if _p not in sys.path and os.path.isdir(_p):
        sys.path.append(_p)

import concourse.bacc as bacc
import concourse.bass as bass
import concourse.tile as tile
from concourse import bass_utils, mybir
from concourse.masks import make_identity

F32 = mybir.dt.float32
F32R = mybir.dt.float32r
F16 = mybir.dt.float16
I16 = mybir.dt.int16

GRID = (64, 64, 100)
V_GRID = GRID[0] * GRID[1] * GRID[2]
MINS = np.array([0.0, 0.0, 0.0], np.float32)
MAXS = np.array([256.0, 256.0, 1.0], np.float32)
H = 4
LN_EPS = 1e-5

P = 128
B, N, DIN, D = 2, 4096, 32, 128
DH = D // H
K27 = 27
NCORES = 8
PTS_CORE = (B * N) // NCORES      # 1024
CHB = 4                            # phase-B chunks per core
PTS_CH = PTS_CORE // CHB           # 256
G2 = PTS_CH // P                   # 2
KG = K27 * G2                      # 54
NI = KG * P                        # 6912 gather rows per chunk
IDXW = NI // 16                    # 432

_cache = {}

# packed fp32 small-weight layout: (name, partitions, cols)
WSPEC = [
    ("fW_aug", 33, 128), ("fW_augT", 128, 33), ("aW", 128, 128),
    ("WqT", 128, 128), ("WkT", 128, 128), ("WvT", 128, 128), ("out_wT", 128, 128),
    ("ab_row", 1, 128), ("bq_row", 1, 128), ("out_b_row", 1, 128),
    ("gamma_row", 1, 128), ("beta_row", 1, 128),
    ("bk_col", 128, 1), ("bv_col", 128, 1), ("pb1_col", 64, 1), ("pb2_col", 128, 1),
    ("eps_col", 128, 1), ("pW1", 3, 64), ("pW2", 64, 128), ("offs_fT", 3, 27),
    ("onehot32", 1, 33), ("ones1f", 1, 128),
]
WOFF = {}
_o = 0
for _n, _pp, _cc in WSPEC:
    WOFF[_n] = _o
    _o += _cc
WTOT = _o
WSPEC16 = [("ones1h", 1, 128), ("headmask", 128, 4)]
WOFF16 = {}
_o = 0
for _n, _pp, _cc in WSPEC16:
    WOFF16[_n] = _o
    _o += _cc
WTOT16 = _o


# ----------------------------------------------------------------- host prep
def _voxelize(coords):
    """Exactly mirror the reference's fp32 voxel hashing."""
    c = coords.astype(np.float32)
    norm = np.clip((c - MINS) / (MAXS - MINS), np.float32(0.0), np.float32(1.0))
    g = np.array(GRID, np.int32)
    vidx = (norm * (g - 1).astype(np.float32)).astype(np.int32)
    vid = (vidx[..., 0] * g[1] + vidx[..., 1]) * g[2] + vidx[..., 2]
    return vidx, vid


def _host_prep(features, coords):
    vidx, vid = _voxelize(coords)

    # --- per-batch sort by voxel id; chunk so no voxel-run spans a chunk ---
    slot_maps = []      # per batch: dict vid -> slot
    chunks = []         # list of (batch, [(sorted_point_indices, r)] per chunk)
    for b in range(B):
        order = np.argsort(vid[b], kind="stable")
        svid = vid[b][order]
        uniq, starts, cnts = np.unique(svid, return_index=True, return_counts=True)
        if cnts.max() > P:
            raise NotImplementedError("voxel with more than 128 points")
        runs = list(zip(uniq.tolist(), starts.tolist(), cnts.tolist()))
        cur = []  # list of runs in current chunk
        cur_n = 0
        bchunks = []
        for u, s, c in runs:
            if cur_n + c > P:
                bchunks.append(cur)
                cur, cur_n = [], 0
            cur.append((u, s, c))
            cur_n += c
        if cur:
            bchunks.append(cur)
        chunks.append((b, bchunks, order))
        slot_maps.append({})

    ca_list = [len(chunks[b][1]) for b in range(B)]
    CA = sum(ca_list)
    CA = -(-CA // 4) * 4  # pad total chunk count to a multiple of 4
    # distribute padding chunks (all-dummy) at the end
    R_TAB = CA * P + 16   # table rows: slots + dummy row at CA*P (rest pad)
    DUMMY = CA * P

    feats_flat = features.reshape(B * N, DIN).astype(np.float32)

    XS = np.zeros((P, CA, DIN + 1), np.float32)
    S = np.zeros((P, CA * P), np.float16)
    c_global = 0
    for b, bchunks, order in chunks:
        for runs in bchunks:
            p0 = 0
            for j, (u, s, c) in enumerate(runs):
                idxs = order[s : s + c]  # original point indices in batch b
                r = np.float16(1.0 / np.float32(c))
                for pi in idxs:
                    XS[p0, c_global, :DIN] = feats_flat[b * N + pi]
                    XS[p0, c_global, DIN] = 1.0
                    S[p0, c_global * P + j] = r
                    p0 += 1
                slot_maps[b][u] = c_global * P + j
            c_global += 1
    XS = XS.reshape(P, CA * (DIN + 1))

    # --- neighbor indices / masks per core ---
    rr = np.arange(-1, 2)
    offs = np.stack(np.meshgrid(rr, rr, rr, indexing="ij"), -1).reshape(-1, 3)
    g = np.array(GRID, np.int32)

    vidx_flat = vidx.reshape(B * N, 3)
    nidx = vidx_flat[:, None, :] + offs[None, :, :]          # (BN, 27, 3)
    inb = np.all((nidx >= 0) & (nidx < g), axis=-1)          # (BN, 27)
    ncl = np.clip(nidx, 0, g - 1)
    nvid = (ncl[..., 0] * g[1] + ncl[..., 1]) * g[2] + ncl[..., 2]

    j_all = np.full((B * N, K27), DUMMY, np.int64)
    valid = np.zeros((B * N, K27), bool)
    for b in range(B):
        sm = slot_maps[b]
        base = b * N
        for i in range(N):
            row = nvid[base + i]
            ib = inb[base + i]
            for k in range(K27):
                if ib[k]:
                    s = sm.get(int(row[k]))
                    if s is not None:
                        j_all[base + i, k] = s
                        valid[base + i, k] = True
    has = valid.any(-1)

    per_core = []
    for core in range(NCORES):
        sl = slice(core * PTS_CORE, (core + 1) * PTS_CORE)
        jc = j_all[sl].reshape(CHB, G2, P, K27)
        # gather order i = (k*G2+g)*P + p  for each chunk
        idx = np.zeros((CHB, K27, G2, P), np.int64)
        for ch in range(CHB):
            for k in range(K27):
                for gg in range(G2):
                    idx[ch, k, gg] = jc[ch, gg, :, k]
        idx = idx.reshape(CHB, NI)
        blocks = []
        for ch in range(CHB):
            blk = np.zeros((16, IDXW), np.int16)
            lin = idx[ch]
            blk[np.arange(NI) % 16, np.arange(NI) // 16] = lin.astype(np.int16)
            blocks.append(np.tile(blk, (8, 1)))
        IDX = np.concatenate(blocks, axis=1)  # (128, CHB*IDXW)

        vc = valid[sl].reshape(CHB, G2, P, K27)
        MASK = np.zeros((P, CHB * KG * H), np.float16)
        for ch in range(CHB):
            for k in range(K27):
                for gg in range(G2):
                    for hh in range(H):
                        MASK[:, (ch * KG + k * G2 + gg) * H + hh] = vc[ch, gg, :, k]
        HAS = np.zeros((P, CHB * G2), np.float32)
        hc = has[sl].reshape(CHB * G2, P)
        for gg in range(CHB * G2):
            HAS[:, gg] = hc[gg]
        XO = np.zeros((DIN + 1, PTS_CORE), np.float32)
        XO[:DIN] = feats_flat[sl].T
        XO[DIN] = 1.0
        per_core.append({"IDX": IDX, "MASK": MASK, "HAS": HAS, "XO": XO})

    return XS, S, CA, R_TAB, per_core


# ------------------------------------------------------------- device program
def _build(CA, R_TAB):
    nc = bacc.Bacc(
        "TRN2",
        target_bir_lowering=False,
        debug=False,
        enable_asserts=False,
        num_devices=1,
    )
    CAP = CA * P
    dt_in = {
        "XS": ([P, CA * (DIN + 1)], F32),
        "S": ([P, CAP], F16),
        "XO": ([DIN + 1, PTS_CORE], F32),
        "IDX": ([P, CHB * IDXW], I16),
        "MASK": ([P, CHB * KG * H], F16),
        "HAS": ([P, CHB * G2], F32),
        "wpack": ([P, WTOT], F32),
        "wpack16": ([P, WTOT16], F16),
    }
    dram = {k: nc.dram_tensor(k, shp, dt, kind="ExternalInput") for k, (shp, dt) in dt_in.items()}
    out_d = nc.dram_tensor("OUT", [P, CHB * G2, D], F32, kind="ExternalOutput")
    table = nc.dram_tensor("tableKV", [R_TAB, 2 * D], F16, kind="Internal")

    from contextlib import ExitStack

    with tile.TileContext(nc) as tc:
        with ExitStack() as ctx, nc.allow_low_precision(
            "fp16 phase-B math; tolerance-checked vs fp32 ref"
        ):
            _emit(ctx, tc, nc, dram, out_d, table, CA)
    nc.compile()
    return nc


def _emit(ctx, tc, nc, dram, out_d, table, CA):
    CAP = CA * P
    W512 = CAP // 512

    const = ctx.enter_context(tc.tile_pool(name="const", bufs=1))

    # ---------- load persistent inputs ----------
    t_in = {}
    for k in ("XO", "IDX", "MASK", "HAS"):
        shp = list(dram[k].shape)
        tl = const.tile(shp, dram[k].dtype, tag=f"in_{k}")
        nc.sync.dma_start(tl[:], dram[k].ap())
        t_in[k] = tl
    wp = const.tile([P, WTOT], F32, tag="wpack")
    nc.sync.dma_start(wp[:], dram["wpack"].ap())
    for nme, pp, cc in WSPEC:
        t_in[nme] = wp[0:pp, WOFF[nme] : WOFF[nme] + cc]
    wp16 = const.tile([P, WTOT16], F16, tag="wpack16")
    nc.sync.dma_start(wp16[:], dram["wpack16"].ap())
    for nme, pp, cc in WSPEC16:
        t_in[nme] = wp16[0:pp, WOFF16[nme] : WOFF16[nme] + cc]

    ident16 = const.tile([P, P], F16, tag="ident16")
    make_identity(nc, ident16[:])
    neg60k = const.tile([P, KG * H], F16, tag="neg60k")
    nc.vector.memset(neg60k[:], -60000.0)

    # fp16 casts of weights (persistent)
    WkT16 = const.tile([D, D], F16, tag="WkT16")
    nc.vector.tensor_copy(WkT16[:], t_in["WkT"][:])
    WvT16 = const.tile([D, D], F16, tag="WvT16")
    nc.vector.tensor_copy(WvT16[:], t_in["WvT"][:])
    owT16 = const.tile([D, D], F16, tag="owT16")
    nc.vector.tensor_copy(owT16[:], t_in["out_wT"][:])
    outb16 = const.tile([1, D], F16, tag="outb16")
    nc.vector.tensor_copy(outb16[:], t_in["out_b_row"][:])

    # persistent phase-A outputs used by phase B
    featR = const.tile([P, CHB * G2, D], F32, tag="featR")
    qR = const.tile([P, CHB * G2, D], F16, tag="qR")
    QPr = const.tile([P, CHB * G2, K27 * H], F16, tag="QPr")
    posVBr = const.tile([K27 * H, D], F16, tag="posVBr")
    gammarep = const.tile([P, D], F32, tag="gammarep")
    betarep = const.tile([P, D], F32, tag="betarep")

    psSm = ctx.enter_context(tc.tile_pool(name="psSm", bufs=2, space="PSUM"))
    psB = ctx.enter_context(tc.tile_pool(name="psB", bufs=2, space="PSUM"))

    # ---------- phase A ----------
    with tc.tile_pool(name="pha", bufs=1) as pha, \
         tc.tile_pool(name="stage", bufs=2) as stage:
        XS32 = pha.tile([P, CA * (DIN + 1)], F32, tag="XS32")
        nc.sync.dma_start(XS32[:], dram["XS"].ap())
        S16 = pha.tile([P, CAP], F16, tag="S16")
        nc.sync.dma_start(S16[:], dram["S"].ap())
        XS16 = pha.tile([P, CA * (DIN + 1)], F16, tag="XS16")
        nc.vector.tensor_copy(XS16[:], XS32[:])

        # W2_aug = fW_aug @ aW (+ ab row); Wq2_aug = fW_aug @ Wq.T (+ bq row)
        W2s16 = pha.tile([DIN + 1, D], F16, tag="W2s16")
        Wq2s = pha.tile([DIN + 1, D], F32, tag="Wq2s")
        for dst, rhs_w, brow in ((W2s16, "aW", "ab_row"), (Wq2s, "WqT", "bq_row")):
            ps = psSm.tile([DIN + 1, D], F32, tag="psSm")
            nc.tensor.matmul(ps[:], t_in["fW_augT"][:], t_in[rhs_w][:], start=True, stop=False)
            nc.tensor.matmul(ps[:], t_in["onehot32"][:], t_in[brow][:], start=False, stop=True)
            nc.vector.tensor_copy(dst[:], ps[:])

        # segment sums (means: 1/cnt folded into S): gsumA[a, u], drains 4-wide
        gsumA = pha.tile([DIN + 1, CAP], F16, tag="gsumA")
        psA = ctx.enter_context(tc.tile_pool(name="psA", bufs=2, space="PSUM"))
        for c4 in range(CA // 4):
            ps = psA.tile([DIN + 1, 512], F32, tag="psA")
            for cc in range(4):
                c = c4 * 4 + cc
                lhs = XS16[:, c * (DIN + 1) : (c + 1) * (DIN + 1)]
                nc.tensor.matmul(
                    ps[:, cc * P : (cc + 1) * P], lhs, S16[:, c * P : (c + 1) * P], start=True, stop=True
                )
            if c4 % 2 == 0:
                nc.vector.tensor_copy(gsumA[:, c4 * 512 : (c4 + 1) * 512], ps[:])
            else:
                nc.scalar.copy(gsumA[:, c4 * 512 : (c4 + 1) * 512], ps[:])

        # grid MLP: gridT16[do, u] = relu(W2_aug.T @ gsumA)
        psW = ctx.enter_context(tc.tile_pool(name="psW", bufs=2, space="PSUM"))
        gridT16 = pha.tile([D, CAP], F16, tag="gridT16")
        for w in range(W512):
            ps = psW.tile([D, 512], F32, tag="psW")
            nc.tensor.matmul(
                ps[:], W2s16[:], gsumA[:, w * 512 : (w + 1) * 512], start=True, stop=True
            )
            nc.scalar.activation(gridT16[:, w * 512 : (w + 1) * 512], ps[:], mybir.ActivationFunctionType.Relu)

        # K/V tables -> DRAM (fused rows [K|V], fp16), 2 chunks per PSUM bank,
        # 4 chunks per staging tile / DMA
        for c4 in range(CA // 4):
            st = stage.tile([P, 4, 2 * D], F16, tag="stage")
            for half in range(2):
                ps = psW.tile([D, 512], F32, tag="psW")
                for q in range(2):
                    c = c4 * 4 + half * 2 + q
                    lhs = gridT16[:, c * P : (c + 1) * P]
                    nc.tensor.matmul(ps[:, q * 256 : q * 256 + D], lhs, WkT16[:], start=True, stop=True)
                    nc.tensor.matmul(ps[:, q * 256 + D : q * 256 + 2 * D], lhs, WvT16[:], start=True, stop=True)
                if half == 0:
                    nc.vector.tensor_copy(st[:, 0:2, :], ps[:].rearrange("p (c e) -> p c e", c=2))
                else:
                    nc.scalar.copy(st[:, 2:4, :], ps[:].rearrange("p (c e) -> p c e", c=2))
            dst = table.ap()[c4 * 4 * P : (c4 * 4 + 4) * P, :]
            dst = dst.rearrange("(c p) e -> p c e", p=P, c=4)
            nc.sync.dma_start(dst, st[:])
        zrow = stage.tile([16, 2 * D], F16, tag="zrow")
        nc.vector.memset(zrow[:], 0.0)
        nc.sync.dma_start(table.ap()[CAP : CAP + 16, :], zrow[:])

        # positional encodings -> posK/posV (k/v input biases folded in)
        ps = psSm.tile([D // 2, K27], F32, tag="psSm")
        nc.tensor.matmul(ps[:], t_in["pW1"][:], t_in["offs_fT"][:], start=True, stop=True)
        h1 = pha.tile([D // 2, K27], F32, tag="h1")
        nc.scalar.activation(h1[:], ps[:], mybir.ActivationFunctionType.Relu, bias=t_in["pb1_col"][:])
        ps = psSm.tile([D, K27], F32, tag="psSm")
        nc.tensor.matmul(ps[:], t_in["pW2"][:], h1[:], start=True, stop=True)
        posT = pha.tile([D, K27], F32, tag="posT")
        nc.scalar.activation(posT[:], ps[:], mybir.ActivationFunctionType.Identity, bias=t_in["pb2_col"][:])

        posKV16 = pha.tile([D, 2, K27], F16, tag="posKV16")
        for i, (wname, bcol) in enumerate((("WkT", "bk_col"), ("WvT", "bv_col"))):
            ps = psSm.tile([D, K27], F32, tag="psSm")
            nc.tensor.matmul(ps[:], t_in[wname][:], posT[:], start=True, stop=True)
            nc.scalar.activation(posKV16[:, i, :], ps[:], mybir.ActivationFunctionType.Identity, bias=t_in[bcol][:])

        # head-blocked positional tables:
        #   posKB[d, k*H+h] = posK[d, k] * (d in head h)   (for q . posK on PE)
        #   posVBr[(k h), e] = posV[e, k] * (e in head h)  (for sum_k eexp*posV on PE)
        posKB = pha.tile([D, K27 * H], F16, tag="posKB")
        posVB = pha.tile([D, K27 * H], F16, tag="posVB")
        for i, dst in enumerate((posKB, posVB)):
            nc.vector.tensor_tensor(
                out=dst[:].rearrange("p (k h) -> p k h", k=K27),
                in0=posKV16[:, i, :].unsqueeze(2).broadcast_to((D, K27, H)),
                in1=t_in["headmask"][:].unsqueeze(1).broadcast_to((D, K27, H)),
                op=mybir.AluOpType.mult,
            )
        ps = psSm.tile([K27 * H, D], F16, tag="psSm")
        nc.tensor.matmul(ps[:], posVB[:], ident16[:], is_transpose=True, start=True, stop=True)
        nc.vector.tensor_copy(posVBr[:], ps[:])

        # qT16 (for QP): qT[dq, i] = Wq2_aug.T @ XO, scaled by 1/sqrt(dh)
        qT16 = pha.tile([D, PTS_CORE], F16, tag="qT16")
        for w in range(PTS_CORE // 512):
            ps = psW.tile([D, 512], F32, tag="psW")
            nc.tensor.matmul(ps[:], Wq2s[:], t_in["XO"][:, w * 512 : (w + 1) * 512], start=True, stop=True)
            nc.scalar.activation(
                qT16[:, w * 512 : (w + 1) * 512], ps[:], mybir.ActivationFunctionType.Copy,
                bias=0.0, scale=float(1.0 / np.sqrt(DH)),
            )
        # QP[i, (k h)] = sum_d qT16[d, i] * posKB[d, (k h)] -> transpose per group
        QPT = pha.tile([K27 * H, PTS_CORE], F16, tag="QPT")
        for w in range(PTS_CORE // 512):
            ps = psW.tile([K27 * H, 512], F32, tag="psW")
            nc.tensor.matmul(ps[:], posKB[:], qT16[:, w * 512 : (w + 1) * 512], start=True, stop=True)
            nc.scalar.copy(QPT[:, w * 512 : (w + 1) * 512], ps[:])
        for gg in range(CHB * G2):
            ps = psSm.tile([P, K27 * H], F16, tag="psSm")
            nc.tensor.matmul(
                ps[:], QPT[:, gg * P : (gg + 1) * P], ident16[0 : K27 * H, 0 : K27 * H],
                is_transpose=True, start=True, stop=True,
            )
            nc.vector.tensor_copy(QPr[:, gg, :], ps[:])

        # feat / q rows for this core's points (2 groups per PSUM bank)
        for pair in range(CHB * G2 // 2):
            psf = psB.tile([P, 2 * D], F32, tag="psB")
            psq = psB.tile([P, 2 * D], F32, tag="psB")
            for q in range(2):
                gg = pair * 2 + q
                xo = t_in["XO"][:, gg * P : (gg + 1) * P]
                nc.tensor.matmul(psf[:, q * D : (q + 1) * D], xo, t_in["fW_aug"][:], start=True, stop=True)
                nc.tensor.matmul(psq[:, q * D : (q + 1) * D], xo, Wq2s[:], start=True, stop=True)
            nc.vector.tensor_copy(featR[:, pair * 2 : pair * 2 + 2, :], psf[:].rearrange("p (g e) -> p g e", g=2))
            nc.scalar.activation(
                qR[:, pair * 2 : pair * 2 + 2, :].rearrange("p g e -> p (g e)"),
                psq[:],
                mybir.ActivationFunctionType.Copy,
                bias=0.0,
                scale=float(1.0 / np.sqrt(DH)),
            )

        for dst, row in ((gammarep, "gamma_row"), (betarep, "beta_row")):
            ps = psSm.tile([P, D], F32, tag="psSm")
            nc.tensor.matmul(ps[:], t_in["ones1f"][:], t_in[row][:], start=True, stop=True)
            nc.vector.tensor_copy(dst[:], ps[:])

    # ---------- phase B ----------
    gpool = ctx.enter_context(tc.tile_pool(name="gpool", bufs=2))
    bpool = ctx.enter_context(tc.tile_pool(name="bpool", bufs=2))
    spool = ctx.enter_context(tc.tile_pool(name="spool", bufs=2))

    for ch in range(CHB):
        G = gpool.tile([P, KG, 2 * D], F16, tag="G")
        nc.gpsimd.dma_gather(
            out_ap=G[:],
            in_ap=table.ap()[:, :],
            idxs_ap=t_in["IDX"][:, ch * IDXW : (ch + 1) * IDXW],
            num_idxs=NI,
            num_idxs_reg=NI,
            elem_size=2 * D,
        )
        kv4 = G[:].rearrange("p (k g) e -> p k g e", k=K27, g=G2)
        # scores from gathered K: prod then tree-reduce over d (within heads)
        prod = bpool.tile([P, K27, G2, D], F16, tag="pv")
        qch = (
            qR[:, ch * G2 : (ch + 1) * G2, :]
            .unsqueeze(1)
            .broadcast_to((P, K27, G2, D))
        )
        nc.vector.tensor_tensor(out=prod[:], in0=kv4[:, :, :, 0:D], in1=qch, op=mybir.AluOpType.mult)
        w = DH // 2
        while w >= 1:
            pr4 = prod[:].rearrange("p k g (h e) -> p (k g) h e", h=H)
            nc.vector.tensor_tensor(
                out=pr4[:, :, :, 0:w],
                in0=pr4[:, :, :, 0:w],
                in1=pr4[:, :, :, w : 2 * w],
                op=mybir.AluOpType.add,
            )
            w //= 2
        # scores = tree result + q.posK term
        scoresR = spool.tile([P, KG * H], F16, tag="scoresR")
        nc.vector.tensor_tensor(
            out=scoresR[:].rearrange("p (k g h) -> p k g h", k=K27, g=G2),
            in0=prod[:, :, :, :].rearrange("p k g (h e) -> p k g h e", h=H)[:, :, :, :, 0],
            in1=QPr[:, ch * G2 : (ch + 1) * G2, :]
            .rearrange("p g (k h) -> p k g h", k=K27),
            op=mybir.AluOpType.add,
        )
        scoresS = spool.tile([P, KG * H], F16, tag="scoresS")
        nc.vector.tensor_copy(scoresS[:], neg60k[:])
        nc.vector.copy_predicated(
            out=scoresS[:],
            mask=t_in["MASK"][:, ch * KG * H : (ch + 1) * KG * H],
            data=scoresR[:],
        )
        # softmax over k (unnormalized; 1/sum applied at the end)
        ghk = lambda t: t[:].rearrange("p (k g h) -> p g h k", k=K27, g=G2, h=H)
        mx = spool.tile([P, G2 * H], F16, tag="mx")
        nc.vector.tensor_reduce(
            out=mx[:].rearrange("p (g h) -> p g h", g=G2),
            in_=ghk(scoresS),
            axis=mybir.AxisListType.X,
            op=mybir.AluOpType.max,
        )
        esub = spool.tile([P, KG * H], F16, tag="esub")
        nc.vector.tensor_tensor(
            out=ghk(esub),
            in0=ghk(scoresS),
            in1=mx[:].rearrange("p (g h) -> p g h", g=G2).unsqueeze(3).broadcast_to((P, G2, H, K27)),
            op=mybir.AluOpType.subtract,
        )
        eexp = spool.tile([P, KG * H], F16, tag="eexp")
        nc.scalar.activation(eexp[:], esub[:], mybir.ActivationFunctionType.Exp)
        ssum = spool.tile([P, G2 * H], F32, tag="ssum")
        nc.vector.tensor_reduce(
            out=ssum[:].rearrange("p (g h) -> p g h", g=G2),
            in_=ghk(eexp),
            axis=mybir.AxisListType.X,
            op=mybir.AluOpType.add,
        )
        sinv = spool.tile([P, G2 * H], F32, tag="sinv")
        nc.vector.reciprocal(sinv[:], ssum[:])
        sinv16 = spool.tile([P, G2 * H], F16, tag="sinv16")
        nc.vector.tensor_copy(sinv16[:], sinv[:])
        # eexp-weighted V, tree-summed over k
        vprod = bpool.tile([P, K27, G2 * D], F16, tag="pv")
        ew = eexp[:].rearrange("p (k g h) -> p k g h", k=K27, g=G2)
        for gg in range(G2):
            nc.vector.tensor_tensor(
                out=vprod[:, :, gg * D : (gg + 1) * D].rearrange("p k (h e) -> p k h e", h=H),
                in0=kv4[:, :, gg, D : 2 * D].rearrange("p k (h e) -> p k h e", h=H),
                in1=ew[:, :, gg, :].unsqueeze(3).broadcast_to((P, K27, H, DH)),
                op=mybir.AluOpType.mult,
            )
        n = K27
        while n > 1:
            c = -(-n // 2)
            m = n - c
            nc.vector.tensor_tensor(
                out=vprod[:, 0:m, :],
                in0=vprod[:, 0:m, :],
                in1=vprod[:, c : c + m, :],
                op=mybir.AluOpType.add,
            )
            n = c
        # per-chunk epilogue
        enh = spool.tile([P, G2, D], F32, tag="enh")
        for gg in range(G2):
            gglob = ch * G2 + gg
            # eexp^T for this group (PE) -> PV = sum_k eexp*posV (PE)
            psE = psSm.tile([K27 * H, P], F16, tag="psSm")
            nc.tensor.matmul(
                psE[:],
                eexp[:].rearrange("p (k g h) -> p k g h", k=K27, g=G2)[:, :, gg, :],
                ident16[:],
                is_transpose=True,
                start=True,
                stop=True,
            )
            eexpT = spool.tile([K27 * H, P], F16, tag="eexpT")
            nc.scalar.copy(eexpT[:], psE[:])
            psPV = psB.tile([P, D], F32, tag="psB")
            nc.tensor.matmul(psPV[:], eexpT[:], posVBr[:], start=True, stop=True)
            # attn_unnorm = vprod_g + PV ; attn = attn_unnorm * sinv (per head)
            att = spool.tile([P, D], F16, tag="att")
            nc.vector.tensor_tensor(
                out=att[:], in0=vprod[:, 0, gg * D : (gg + 1) * D], in1=psPV[:], op=mybir.AluOpType.add
            )
            attn = spool.tile([P, D], F16, tag="attn")
            nc.vector.tensor_tensor(
                out=attn[:].rearrange("p (h e) -> p h e", h=H),
                in0=att[:].rearrange("p (h e) -> p h e", h=H),
                in1=sinv16[:, gg * H : (gg + 1) * H].unsqueeze(2).broadcast_to((P, H, DH)),
                op=mybir.AluOpType.mult,
            )
            psT = psSm.tile([P, D], F16, tag="psSm")
            nc.tensor.matmul(psT[:], attn[:], ident16[:], is_transpose=True, start=True, stop=True)
            attnT = spool.tile([P, D], F16, tag="attnT")
            nc.scalar.copy(attnT[:], psT[:])
            psO = psB.tile([P, D], F32, tag="psB")
            nc.tensor.matmul(psO[:], attnT[:], owT16[:], start=True, stop=False)
            nc.tensor.matmul(psO[:], t_in["ones1h"][:], outb16[:], start=False, stop=True)
            nc.vector.scalar_tensor_tensor(
                out=enh[:, gg, :],
                in0=psO[:],
                scalar=t_in["HAS"][:, gglob : gglob + 1],
                in1=featR[:, gglob, :],
                op0=mybir.AluOpType.mult,
                op1=mybir.AluOpType.add,
            )
        # layernorm, both groups at once
        mu = spool.tile([P, G2], F32, tag="mu")
        nc.vector.tensor_reduce(out=mu[:], in_=enh[:], axis=mybir.AxisListType.X, op=mybir.AluOpType.add)
        mus = spool.tile([P, G2], F32, tag="mus")
        nc.vector.tensor_scalar_mul(mus[:], mu[:], float(1.0 / D))
        cent = spool.tile([P, G2, D], F32, tag="cent")
        nc.vector.tensor_tensor(
            out=cent[:], in0=enh[:], in1=mus[:].unsqueeze(2).broadcast_to((P, G2, D)), op=mybir.AluOpType.subtract
        )
        sq = spool.tile([P, G2, D], F32, tag="sq")
        nc.vector.tensor_tensor(out=sq[:], in0=cent[:], in1=cent[:], op=mybir.AluOpType.mult)
        var = spool.tile([P, G2], F32, tag="var")
        nc.vector.tensor_reduce(out=var[:], in_=sq[:], axis=mybir.AxisListType.X, op=mybir.AluOpType.add)
        varm = spool.tile([P, G2], F32, tag="varm")
        nc.vector.tensor_scalar_mul(varm[:], var[:], float(1.0 / D))
        sd = spool.tile([P, G2], F32, tag="sd")
        nc.scalar.activation(sd[:], varm[:], mybir.ActivationFunctionType.Sqrt, bias=t_in["eps_col"][:, 0:1])
        rinv = spool.tile([P, G2], F32, tag="rinv")
        nc.vector.reciprocal(rinv[:], sd[:])
        y1 = spool.tile([P, G2, D], F32, tag="y1")
        nc.vector.tensor_tensor(
            out=y1[:], in0=cent[:], in1=rinv[:].unsqueeze(2).broadcast_to((P, G2, D)), op=mybir.AluOpType.mult
        )
        y2 = spool.tile([P, G2, D], F32, tag="y2")
        nc.vector.tensor_tensor(
            out=y2[:], in0=y1[:], in1=gammarep[:].unsqueeze(1).broadcast_to((P, G2, D)), op=mybir.AluOpType.mult
        )
        y3 = spool.tile([P, G2, D], F32, tag="y3")
        nc.vector.tensor_tensor(
            out=y3[:], in0=y2[:], in1=betarep[:].unsqueeze(1).broadcast_to((P, G2, D)), op=mybir.AluOpType.add
        )
        nc.sync.dma_start(out_d.ap()[:, ch * G2 : (ch + 1) * G2, :], y3[:])


# ----------------------------------------------------------------- entry point
def build_in_maps(inputs, XS, S, per_core):
    in_w = np.asarray(inputs["in_w"], np.float32)
    in_b = np.asarray(inputs["in_b"], np.float32)
    rr = np.arange(-1, 2)
    offs = np.stack(np.meshgrid(rr, rr, rr, indexing="ij"), -1).reshape(-1, 3)

    onehot32 = np.zeros((1, DIN + 1), np.float32)
    onehot32[0, DIN] = 1.0
    fW_aug = np.concatenate([inputs["fW"], np.asarray(inputs["fb"])[None, :]], 0).astype(np.float32)
    vals = {
        "fW_aug": fW_aug,
        "fW_augT": fW_aug.T.copy(),
        "aW": np.asarray(inputs["aW"], np.float32),
        "ab_row": np.asarray(inputs["ab"], np.float32)[None, :],
        "WqT": in_w[:D].T.copy(),
        "WkT": in_w[D : 2 * D].T.copy(),
        "WvT": in_w[2 * D :].T.copy(),
        "out_wT": np.asarray(inputs["out_w"], np.float32).T.copy(),
        "bq_row": in_b[:D][None, :].copy(),
        "bk_col": in_b[D : 2 * D][:, None].copy(),
        "bv_col": in_b[2 * D :][:, None].copy(),
        "out_b_row": np.asarray(inputs["out_b"], np.float32)[None, :],
        "pW1": np.asarray(inputs["pW1"], np.float32),
        "pb1_col": np.asarray(inputs["pb1"], np.float32)[:, None],
        "pW2": np.asarray(inputs["pW2"], np.float32),
        "pb2_col": np.asarray(inputs["pb2"], np.float32)[:, None],
        "offs_fT": offs.T.astype(np.float32),
        "gamma_row": np.asarray(inputs["gamma"], np.float32)[None, :],
        "beta_row": np.asarray(inputs["beta"], np.float32)[None, :],
        "eps_col": np.full((D, 1), LN_EPS, np.float32),
        "onehot32": onehot32,
        "ones1f": np.ones((1, P), np.float32),
    }
    wpack = np.zeros((P, WTOT), np.float32)
    for nme, pp, cc in WSPEC:
        wpack[0:pp, WOFF[nme] : WOFF[nme] + cc] = vals[nme]
    vals16 = {
        "ones1h": np.ones((1, P), np.float16),
        "headmask": (np.arange(D)[:, None] // DH == np.arange(H)[None, :]).astype(np.float16),
    }
    wpack16 = np.zeros((P, WTOT16), np.float16)
    for nme, pp, cc in WSPEC16:
        wpack16[0:pp, WOFF16[nme] : WOFF16[nme] + cc] = vals16[nme]

    rep = {"XS": XS, "S": S, "wpack": wpack, "wpack16": wpack16}
    in_maps = []
    for core in range(NCORES):
        m = dict(rep)
        m["XO"] = per_core[core]["XO"]
        m["IDX"] = per_core[core]["IDX"]
        m["MASK"] = per_core[core]["MASK"]
        m["HAS"] = per_core[core]["HAS"]
        in_maps.append(m)
    return in_maps


def kernel(**inputs):
    features = np.asarray(inputs["features"], np.float32)
    coords = np.asarray(inputs["coords"], np.float32)

    XS, S, CA, R_TAB, per_core = _host_prep(features, coords)

    key = (CA, R_TAB)
    if key not in _cache:
        _cache[key] = _build(CA, R_TAB)
    nc = _cache[key]

    in_maps = build_in_maps(inputs, XS, S, per_core)
    res = bass_utils.run_bass_kernel_spmd(nc, in_maps, core_ids=list(range(NCORES)))

    out = np.zeros((B * N, D), np.float32)
    for core in range(NCORES):
        o = res.results[core]["OUT"]  # (P, CHB*G2, D)
        for gg in range(CHB * G2):
            out[core * PTS_CORE + gg * P : core * PTS_CORE + (gg + 1) * P] = o[:, gg, :]
    return out.reshape(B, N, D)
